# revision 27
# baseline (speedup 1.0000x reference)
"""Trainium2 Bass kernel for nn_AttMatch (2-graph attention + SAGEConv GNN).

Self-contained: takes the full unsharded inputs of the reference problem,
shards across 8 NeuronCores internally, runs one SPMD NEFF, and gathers the
full [8192, 8192] sigmoid adjacency output.

Sharding (v5): query-sharded attention -- each core owns a 512-node slice of
EACH graph; keys/values cover the full 8192-row concatenated target set on
every core, so the softmax over targets is fully local (no AllReduce).

Collectives per layer (AllGather only, warmed up at start):
  A: out(g0)
  B (l=0): k|v|y own-slices for layer 1 (g0, computed with layer-1 weights
           right after h(g0)) merged with out(g1)    -> layer 1 needs no
  C (l=0): k|v|y own-slices (g1)                        projections at all
  B (l=1): h(g0) | out(g1)   (final-phase columns)
  C (l=1): h(g1)

SAGEConv is applied once per (layer, graph) via the algebraic merge
  h = relu( M @ (X@Wls - out@Wl1) + X_own@Wrs - out_own@Wr1 + bl ),
with M^T column shards loaded once (M is layer-invariant).  The g0 SAGE tail
is interleaved into attention(g1); the g1 tail is carried into the next
phase's PE stream.  The attention inner loop runs with a 2-pair lookahead so
the in-order PE queue never waits on exp or the epilogue.  Output in fp16.
"""

import numpy as np
import ml_dtypes

import concourse.bass as bass
import concourse.bacc as bacc
import concourse.tile as tile
import concourse.mybir as mybir
from concourse.bass_utils import run_bass_kernel_spmd

BF16 = ml_dtypes.bfloat16

N = 4096           # nodes per graph
D = 128            # feature dim (in == out == 128)
NCORES = 8
SH = N // NCORES   # 512-node own slice per graph per core
T = 2 * N          # concatenated target set
NTT = T // 128     # 64 target tiles
NPAIR = NTT // 2   # 32 pairs of target tiles (one exp per pair)
NJ = N // 128      # 32 source-node tiles per graph
INV_SCALE = 1.0 / np.sqrt(128.0)

F32 = mybir.dt.float32
BF = mybir.dt.bfloat16
FP16 = mybir.dt.float16

ADD = mybir.AluOpType.add
MULT = mybir.AluOpType.mult
MAX = mybir.AluOpType.max

# wm indices (per layer l: base = 7*l)
WK, WQ, WV, WLS, WL1N, WRS, WR1N = range(7)
# bias indices (per layer l: base = 4*l)
BK, BQ, BV, BL = range(4)

_cache = {}


def _build_nc():
    nc = bacc.Bacc("TRN2", target_bir_lowering=False, debug=False,
                   num_devices=NCORES)

    # ---- external I/O ----
    x1t = nc.dram_tensor("x1t", [D, N], BF, kind="ExternalInput")
    x2t = nc.dram_tensor("x2t", [D, N], BF, kind="ExternalInput")
    xgt_in = [x1t, x2t]
    xown_in = nc.dram_tensor("xown", [2, D, SH], BF, kind="ExternalInput")
    mtc_in = [nc.dram_tensor("mtc1", [NJ, 128, SH], BF, kind="ExternalInput"),
              nc.dram_tensor("mtc2", [NJ, 128, SH], BF, kind="ExternalInput")]
    wm_in = nc.dram_tensor("wm", [14, 128, 128], BF, kind="ExternalInput")
    bs_in = nc.dram_tensor("bs", [8, 128, 1], F32, kind="ExternalInput")
    out_ext = nc.dram_tensor("out", [2, SH, T], FP16, kind="ExternalOutput")

    # ---- internal DRAM for collectives ----
    rg = [list(range(NCORES))]
    ago_in = [nc.dram_tensor(f"ago_in_{l}", [D, SH], BF) for l in range(2)]
    ago_out = [nc.dram_tensor(f"ago_out_{l}", [NCORES, D, SH], BF,
                              addr_space="Shared") for l in range(2)]
    # AG-B payloads: l=0 -> [k0, v0, y0, out1]; l=1 -> [h0, out1]
    agb_n = [4, 2]
    agb_in = [nc.dram_tensor(f"agb_in_{l}", [agb_n[l], D, SH], BF)
              for l in range(2)]
    agb_out = [nc.dram_tensor(f"agb_out_{l}", [NCORES, agb_n[l], D, SH], BF,
                              addr_space="Shared") for l in range(2)]
    # AG-C payloads: l=0 -> [k1, v1, y1]; l=1 -> [h1]
    agc_n = [3, 1]
    agc_in = [nc.dram_tensor(f"agc_in_{l}", [agc_n[l], D, SH], BF)
              for l in range(2)]
    agc_out = [nc.dram_tensor(f"agc_out_{l}", [NCORES, agc_n[l], D, SH], BF,
                              addr_space="Shared") for l in range(2)]

    with tile.TileContext(nc) as tc:
        with (
            tc.tile_pool(name="const", bufs=1) as cpool,
            tc.tile_pool(name="xt", bufs=1) as xt_pool,
            tc.tile_pool(name="xo", bufs=2) as xo_pool,
            tc.tile_pool(name="kv", bufs=1) as kv_pool,
            tc.tile_pool(name="qt", bufs=1) as qt_pool,
            tc.tile_pool(name="es", bufs=5) as es_pool,
            tc.tile_pool(name="cs", bufs=2) as cs_pool,
            tc.tile_pool(name="st", bufs=2) as st_pool,
            tc.tile_pool(name="kvo", bufs=1) as kvo_pool,
            tc.tile_pool(name="y", bufs=1) as y_pool,
            tc.tile_pool(name="ub", bufs=1) as ub_pool,
            tc.tile_pool(name="of", bufs=1) as of_pool,
            tc.tile_pool(name="mt", bufs=16) as mt_pool,
            tc.tile_pool(name="z", bufs=3) as z_pool,
            tc.tile_pool(name="pa", bufs=2, space="PSUM") as pa_pool,
            tc.tile_pool(name="pb", bufs=1, space="PSUM") as pb_pool,
            tc.tile_pool(name="pc", bufs=2, space="PSUM") as pc_pool,
        ):
            # ---- constants ----
            wm = cpool.tile([128, 14 * 128], BF, name="wm_sb")
            nc.sync.dma_start(
                wm.rearrange("p (i f) -> p i f", i=14),
                wm_in.ap().rearrange("i p f -> p i f"))
            bs = cpool.tile([128, 8], F32, name="bs_sb")
            nc.sync.dma_start(
                bs.rearrange("p (i f) -> p i f", i=8),
                bs_in.ap().rearrange("i p f -> p i f"))
            ones_m1 = cpool.tile([128, 1], BF, name="ones_m1")
            nc.vector.memset(ones_m1[:], 1.0)
            ones_c = cpool.tile([1, 128], BF, name="ones_c")
            nc.vector.memset(ones_c[:], 1.0)

            def W(l, i):
                base = 7 * l + i
                return wm[:, 128 * base:128 * (base + 1)]

            def B(l, i):
                return bs[:, 4 * l + i:4 * l + i + 1]

            # warm up the collective rings FIRST (before any bulk load) so
            # the first-CC cold-start / inter-core skew is absorbed as early
            # as possible
            wc_in = nc.dram_tensor("wc_in", [128, 8], BF)
            wc_out = nc.dram_tensor("wc_out", [NCORES, 128, 8], BF,
                                    addr_space="Shared")
            nc.sync.dma_start(wc_in.ap(), wm[:, 0:8])
            nc.gpsimd.collective_compute(
                "AllGather", mybir.AluOpType.bypass, replica_groups=rg,
                ins=[wc_in.ap()], outs=[wc_out.ap()])

            # ---- generation-0 node features (split loads: compute starts
            # after the first chunk) ----
            xg = []
            for g in range(2):
                t = xt_pool.tile([D, N], BF, name=f"x{g}t_0", tag=f"xt{g}")
                for hh in range(2):
                    nc.sync.dma_start(t[:, hh * 2048:(hh + 1) * 2048],
                                      xgt_in[g][:, hh * 2048:(hh + 1) * 2048])
                xg.append(t)
            xo = []
            for g in range(2):
                t = xo_pool.tile([D, SH], BF, name=f"xown{g}_0", tag=f"xo{g}")
                nc.sync.dma_start(t[:], xown_in[g])
                xo.append(t)

            # M^T column shards: identical for both layers -- load once and
            # keep resident (16 tiles, never rewritten)
            mtcs = [[], []]
            for g in range(2):
                for jb in range(8):
                    mtc_t = mt_pool.tile([128, 4 * SH], BF, tag="mt",
                                         name=f"mtc_{g}_{jb}")
                    nc.sync.dma_start(
                        mtc_t.rearrange("p (j n) -> p j n", j=4),
                        mtc_in[g].ap()[4 * jb:4 * jb + 4]
                        .rearrange("j p n -> p j n"))
                    mtcs[g].append(mtc_t)

            hown_final = None
            xgt2 = [None, None]
            carry = []          # g1 SAGE tail of the previous layer

            for l in range(2):
                kt = kv_pool.tile([D, T], BF, name=f"kt_{l}", tag="kt")
                vn = kv_pool.tile([128, T], BF, name=f"vn_{l}", tag="vn")
                ybig = [y_pool.tile([128, N], BF, name=f"y_{l}_{g}",
                                    tag=f"y{g}") for g in range(2)]
                qt = [None, None]
                csa = [None, None]
                php = [None, None]
                outt = [None, None]
                psa = [None, None]
                hT = [None, None]
                xg_next = [None, None]
                pend = {0: [], 1: []}
                xg_l = xg
                xo_l = xo

                def proj_chunk(h, c, l=l, kt=kt, vn=vn, xg_l=xg_l):
                    """k^T and natural-layout v tiles, one 512-wide chunk of
                    target half h (layer 0 only; layer 1 gets k/v/y via AG)."""
                    ps = pc_pool.tile([128, 512], F32, tag="c",
                                      name=f"psk_{l}_{h}_{c}")
                    nc.tensor.matmul(ps[:], W(l, WK),
                                     xg_l[h][:, c * 512:(c + 1) * 512],
                                     start=True, stop=True)
                    nc.vector.tensor_scalar(
                        kt[:, h * N + c * 512:h * N + (c + 1) * 512],
                        ps[:], B(l, BK), None, ADD)
                    psv = pc_pool.tile([128, 512], F32, tag="c",
                                       name=f"psv_{l}_{h}_{c}")
                    for k in range(4):
                        t0 = c * 4 + k
                        nc.tensor.matmul(
                            psv[:, k * 128:(k + 1) * 128],
                            xg_l[h][:, t0 * 128:(t0 + 1) * 128],
                            W(l, WV), start=True, stop=True)
                    # v bias folded in post-softmax (sum(alpha)=1)
                    nc.vector.tensor_copy(
                        vn[:, h * N + c * 512:h * N + (c + 1) * 512],
                        psv[:])

                def gather_kvy(h, l=l, kt=kt, vn=vn, ybig=ybig):
                    """Layer 1: k/v/y for target half h arrive via AG-B/C of
                    layer 0 (computed there with layer-1 weights)."""
                    src = agb_out[0].ap() if h == 0 else agc_out[0].ap()
                    for i, dst in ((0, kt[:, h * N:(h + 1) * N]),
                                   (1, vn[:, h * N:(h + 1) * N]),
                                   (2, ybig[h][:])):
                        nc.gpsimd.dma_start(
                            dst.rearrange("p (c n) -> p c n", c=NCORES),
                            src[:, i].rearrange("c p n -> p c n"))

                def proj_q(g, l=l, qt=qt, xo_l=xo_l):
                    ps = pc_pool.tile([128, 512], F32, tag="c",
                                      name=f"psq_{l}_{g}")
                    nc.tensor.matmul(ps[:], W(l, WQ), xo_l[g][:],
                                     start=True, stop=True)
                    q = qt_pool.tile([D, SH], BF, name=f"qt_{l}_{g}",
                                     tag=f"qt{g}")
                    nc.vector.tensor_scalar(q[:], ps[:], B(l, BQ), None, ADD)
                    qt[g] = q

                def yfill(g, jb, l=l, ybig=ybig, xg_l=xg_l):
                    """Y = X @ Wls, natural layout, 4 node tiles (layer 0)."""
                    psy = pc_pool.tile([128, 512], F32, tag="c",
                                       name=f"psy_{l}_{g}_{jb}")
                    for k in range(4):
                        jt = jb * 4 + k
                        nc.tensor.matmul(psy[:, k * 128:(k + 1) * 128],
                                         xg_l[g][:, jt * 128:(jt + 1) * 128],
                                         W(l, WLS), start=True, stop=True)
                    nc.vector.tensor_copy(
                        ybig[g][:, jb * 512:(jb + 1) * 512], psy[:])

                def attn_scores(g, p, l=l, kt=kt, qt=qt, pend=pend):
                    ps_s = pa_pool.tile([128, 1024], F32, tag="ps_s",
                                        name=f"pss_{l}_{g}_{p}")
                    for i in range(2):
                        tt = 2 * p + i
                        nc.tensor.matmul(ps_s[:, i * 512:(i + 1) * 512],
                                         kt[:, tt * 128:(tt + 1) * 128],
                                         qt[g][:], start=True, stop=True)
                    es = es_pool.tile([128, 1024], BF, tag="es",
                                      name=f"es_{l}_{g}_{p}")
                    nc.scalar.activation(es[:], ps_s[:],
                                         mybir.ActivationFunctionType.Exp,
                                         scale=INV_SCALE)
                    pend[g].append((p, es))

                def attn_value(g, l=l, vn=vn, php=php, csa=csa, pend=pend):
                    p, es = pend[g].pop(0)
                    for i in range(2):
                        tt = 2 * p + i
                        nc.tensor.matmul(php[g][:],
                                         vn[:, tt * 128:(tt + 1) * 128],
                                         es[:, i * 512:(i + 1) * 512],
                                         start=(tt == 0), stop=(tt == NTT - 1))
                    if p == 0:
                        nc.vector.tensor_copy(csa[g][:], es[:])
                    else:
                        nc.vector.tensor_tensor(csa[g][:], csa[g][:], es[:],
                                                ADD)

                def epi(g, l=l, csa=csa, php=php, outt=outt):
                    """softmax epilogue: outT = php / colsum + bv.

                    g0 publishes via AG-A; g1 stages into the AG-B payload
                    (published together with the g0 tail)."""
                    csf = st_pool.tile([128, 512], BF, tag="csf",
                                       name=f"csf_{l}_{g}")
                    nc.vector.tensor_tensor(csf[:], csa[g][:, 0:512],
                                            csa[g][:, 512:1024], ADD)
                    pcs = pc_pool.tile([1, 512], F32, tag="c",
                                       name=f"pcs_{l}_{g}")
                    nc.tensor.matmul(pcs[:], ones_m1[:], csf[:],
                                     start=True, stop=True)
                    csb = st_pool.tile([1, 512], BF, tag="csb",
                                       name=f"csb_{l}_{g}")
                    nc.vector.tensor_copy(csb[:], pcs[:])
                    ps_rep = pc_pool.tile([128, 512], F32, tag="c",
                                          name=f"psrep_{l}_{g}")
                    nc.tensor.matmul(ps_rep[:], ones_c[:], csb[:],
                                     start=True, stop=True)
                    rr = st_pool.tile([128, 512], F32, tag="rr",
                                      name=f"rr_{l}_{g}")
                    nc.vector.reciprocal_approx_fast(rr[:], ps_rep[:])
                    pre = st_pool.tile([128, 512], F32, tag="pre",
                                       name=f"pre_{l}_{g}")
                    nc.vector.tensor_tensor(pre[:], php[g][:], rr[:], MULT)
                    ot = st_pool.tile([D, SH], BF, tag=f"ot{g}",
                                      name=f"outt_{l}_{g}")
                    nc.vector.tensor_scalar(ot[:], pre[:], B(l, BV), None,
                                            ADD)
                    outt[g] = ot
                    if g == 0:
                        nc.gpsimd.dma_start(ago_in[l][:], ot[:])
                        nc.gpsimd.collective_compute(
                            "AllGather", mybir.AluOpType.bypass,
                            replica_groups=rg,
                            ins=[ago_in[l][:]], outs=[ago_out[l][:]])
                    else:
                        nc.gpsimd.dma_start(
                            agb_in[l].ap()[agb_n[l] - 1], ot[:])

                def gather_of(g, l=l):
                    # post-collective gathers go on the gpsimd queue so the
                    # CC-semaphore wait never blocks the sync DMA queue
                    of = of_pool.tile([D, N], BF, tag="of",
                                      name=f"of_{l}_{g}")
                    src = (ago_out[l].ap() if g == 0
                           else agb_out[l].ap()[:, agb_n[l] - 1])
                    nc.gpsimd.dma_start(
                        of.rearrange("p (c n) -> p c n", c=NCORES),
                        src.rearrange("c p n -> p c n"))
                    return of

                def gather_x(g, l=l):
                    # layer 1 only: the full h (= final-phase F^T columns)
                    t = xt_pool.tile([D, N], BF, name=f"x{g}t_{l + 1}",
                                     tag=f"xt{g}")
                    src = (agb_out[l].ap()[:, 0] if g == 0
                           else agc_out[l].ap()[:, 0])
                    nc.gpsimd.dma_start(
                        t.rearrange("p (c n) -> p c n", c=NCORES),
                        src.rearrange("c p n -> p c n"))
                    return t

                def ufill(g, ofl, ub, jb, l=l):
                    """U' = out @ (-Wl1), natural layout, 4 node tiles."""
                    psu = pc_pool.tile([128, 512], F32, tag="c",
                                       name=f"psu_{l}_{g}_{jb}")
                    for k in range(4):
                        jt = jb * 4 + k
                        nc.tensor.matmul(psu[:, k * 128:(k + 1) * 128],
                                         ofl[0][:, jt * 128:(jt + 1) * 128],
                                         W(l, WL1N), start=True, stop=True)
                    nc.vector.tensor_copy(ub[:, jb * 512:(jb + 1) * 512],
                                          psu[:])

                def vsub(g, ub, ybig=ybig):
                    # V = Y + U'  (in place over ybig)
                    nc.vector.tensor_tensor(ybig[g][:], ybig[g][:], ub[:],
                                            ADD)

                def mfill(g, jb, l=l, psa=psa, ybig=ybig):
                    if psa[g] is None:
                        psa[g] = pb_pool.tile([128, 512], F32, tag="psa",
                                              name=f"psa_{l}_{g}")
                    for k in range(4):
                        jt = jb * 4 + k
                        nc.tensor.matmul(psa[g][:],
                                         ybig[g][:, jt * 128:(jt + 1) * 128],
                                         mtcs[g][jb][:, k * SH:(k + 1) * SH],
                                         start=(jt == 0), stop=False)

                def ownparts(g, stage, l=l, hT=hT):
                    """Layer 0 tails: produce layer-1 k/v/y for the own node
                    slice (from h) and stage them into the AG payload."""
                    h = hT[g]
                    psk = pc_pool.tile([128, 512], F32, tag="c",
                                       name=f"psok_{l}_{g}")
                    nc.tensor.matmul(psk[:], W(1, WK), h[:],
                                     start=True, stop=True)
                    ko = kvo_pool.tile([D, SH], BF, tag=f"ko{g}",
                                       name=f"ko_{l}_{g}")
                    nc.vector.tensor_scalar(ko[:], psk[:], B(1, BK), None,
                                            ADD)
                    psv = pc_pool.tile([128, 512], F32, tag="c",
                                       name=f"psov_{l}_{g}")
                    for k in range(4):
                        nc.tensor.matmul(psv[:, k * 128:(k + 1) * 128],
                                         h[:, k * 128:(k + 1) * 128],
                                         W(1, WV), start=True, stop=True)
                    vo = kvo_pool.tile([128, SH], BF, tag=f"vo{g}",
                                       name=f"vo_{l}_{g}")
                    nc.vector.tensor_copy(vo[:], psv[:])
                    psy = pc_pool.tile([128, 512], F32, tag="c",
                                       name=f"psoy_{l}_{g}")
                    for k in range(4):
                        nc.tensor.matmul(psy[:, k * 128:(k + 1) * 128],
                                         h[:, k * 128:(k + 1) * 128],
                                         W(1, WLS), start=True, stop=True)
                    yo = kvo_pool.tile([128, SH], BF, tag=f"yo{g}",
                                       name=f"yo_{l}_{g}")
                    nc.vector.tensor_copy(yo[:], psy[:])
                    for i, t in ((0, ko), (1, vo), (2, yo)):
                        nc.gpsimd.dma_start(stage.ap()[i], t[:])

                def tail_end(g, l=l, psa=psa, outt=outt, hT=hT, xo_l=xo_l,
                             ownparts=ownparts):
                    """Wr terms + bias(+relu); publish AG-B (g0) / AG-C (g1)."""
                    nc.tensor.matmul(psa[g][:], W(l, WRS), xo_l[g][:],
                                     start=False, stop=False)
                    nc.tensor.matmul(psa[g][:], W(l, WR1N), outt[g][:],
                                     start=False, stop=True)
                    h = xo_pool.tile([D, SH], BF, name=f"hown_{l}_{g}",
                                     tag=f"xo{g}")
                    if l == 0:
                        nc.vector.tensor_scalar(h[:], psa[g][:], B(l, BL),
                                                0.0, ADD, MAX)
                    else:
                        nc.vector.tensor_scalar(h[:], psa[g][:], B(l, BL),
                                                None, ADD)
                    hT[g] = h
                    stage = agb_in[l] if g == 0 else agc_in[l]
                    if l == 0:
                        ownparts(g, stage)
                    else:
                        nc.gpsimd.dma_start(stage.ap()[0], h[:])
                    src_t = agb_in[l] if g == 0 else agc_in[l]
                    dst_t = agb_out[l] if g == 0 else agc_out[l]
                    nc.gpsimd.collective_compute(
                        "AllGather", mybir.AluOpType.bypass,
                        replica_groups=rg,
                        ins=[src_t[:]], outs=[dst_t[:]])

                # ================= layer emission =================
                php[0] = pb_pool.tile([128, 512], F32, tag="php",
                                      name=f"php_{l}_0")
                csa[0] = cs_pool.tile([128, 1024], BF, tag="csa",
                                      name=f"csa_{l}_0")
                csa[1] = cs_pool.tile([128, 1024], BF, tag="csa",
                                      name=f"csa_{l}_1")

                if l == 0:
                    for c in range(8):
                        proj_chunk(0, c)
                    proj_q(0)
                    for jb in range(8):
                        yfill(0, jb)
                    # second-half projections interleave into pairs 6..13
                    ins2 = []
                    for c in range(8):
                        ins2.append(lambda c=c: proj_chunk(1, c))
                        ins2.append(lambda c=c: yfill(1, c))
                else:
                    gather_kvy(0)
                    proj_q(0)
                    ins2 = []

                # attention(g0): previous layer's g1 SAGE tail interleaved
                # (3 items/pair -- its deps precede this layer's start), plus
                # the half-1 projections for l=0
                for p in range(NPAIR):
                    attn_scores(0, p)
                    if len(pend[0]) > 2:
                        attn_value(0)
                    for _ in range(3):
                        if carry:
                            carry.pop(0)()
                    if p >= 6:
                        for _ in range(2):
                            if ins2:
                                ins2.pop(0)()
                    if p == 15:
                        proj_q(1)   # xo[1]: external (l0) / carried h (l1)
                        if l == 1:
                            gather_kvy(1)
                while pend[0]:
                    attn_value(0)
                epi(0)

                # attention(g1) with the g0 SAGE tail interleaved
                php[1] = pb_pool.tile([128, 512], F32, tag="php",
                                      name=f"php_{l}_1")
                fill = []
                of0 = [None]
                ub0 = ub_pool.tile([128, N], BF, tag="ub", name=f"ub_{l}_0")

                def mk_uf(jb):
                    return lambda: ufill(0, of0, ub0, jb)

                def mk_mf(jb):
                    return lambda: mfill(0, jb)

                for jb in range(8):
                    fill.append(mk_uf(jb))
                fill.append(lambda: vsub(0, ub0))
                for jb in range(8):
                    fill.append(mk_mf(jb))

                for p in range(NPAIR):
                    attn_scores(1, p)
                    if len(pend[1]) > 2:
                        attn_value(1)
                    if p == 12:
                        of0[0] = gather_of(0)
                    if p >= 15 and fill:
                        fill.pop(0)()
                while pend[1]:
                    attn_value(1)
                epi(1)
                while fill:
                    fill.pop(0)()
                tail_end(0)                       # issues AG-B
                if l == 1:
                    xg_next[0] = gather_x(0)
                of1 = [gather_of(1)]

                # g1 SAGE tail -> carried into the next phase's PE stream.
                # NOTE: bind every per-layer name via default args -- the
                # loop body is one scope and rebinds them next iteration.
                ub1 = ub_pool.tile([128, N], BF, tag="ub", name=f"ub_{l}_1")

                def mk_uf1(jb, f=ufill, o=of1, u=ub1):
                    return lambda: f(1, o, u, jb)

                def mk_mf1(jb, f=mfill):
                    return lambda: f(1, jb)

                carry = []
                for jb in range(8):
                    carry.append(mk_uf1(jb))
                carry.append(lambda f=vsub, u=ub1: f(1, u))
                for jb in range(8):
                    carry.append(mk_mf1(jb))
                carry.append(lambda f=tail_end: f(1))
                if l == 1:
                    carry.append(
                        lambda xn=xg_next, f=gather_x: xn.__setitem__(
                            1, f(1)))

                if l == 0:
                    xo = hT
                else:
                    hown_final = hT
                    xgt2 = xg_next

            # ---- final adjacency: sigmoid(F @ F^T), own 1024 rows ----
            def final_block(ch, grow, rt, cp):
                lhs = hown_final[grow][:, rt * 128:(rt + 1) * 128]
                ps_z = pa_pool.tile([128, 1024], F32, tag="ps_s",
                                    name=f"psz_{ch}_{grow}_{rt}_{cp}")
                for i in range(2):
                    c0 = cp * 1024 + i * 512
                    nc.tensor.matmul(
                        ps_z[:, i * 512:(i + 1) * 512], lhs,
                        xgt2[ch][:, c0:c0 + 512],
                        start=True, stop=True)
                z = z_pool.tile([128, 1024], FP16, tag="z",
                                name=f"z_{ch}_{grow}_{rt}_{cp}")
                nc.scalar.activation(
                    z[:], ps_z[:], mybir.ActivationFunctionType.Sigmoid)
                nc.sync.dma_start(
                    out_ext[grow, rt * 128:(rt + 1) * 128,
                            ch * N + cp * 1024:ch * N + (cp + 1) * 1024],
                    z[:])

            def final_step(ch, grow, rt, cp):
                final_block(ch, grow, rt, cp)
                for _ in range(3):
                    if carry:
                        carry.pop(0)()

            blocks = [(ch, grow, rt, cp) for ch in range(2)
                      for grow in range(2) for rt in range(4)
                      for cp in range(4)]
            for blk in blocks:
                final_step(*blk)

    nc.compile()
    return nc


def _host_prep(inputs):
    """Build per-core input maps from the full problem inputs."""
    x1 = np.asarray(inputs["x1"], np.float32)
    x2 = np.asarray(inputs["x2"], np.float32)
    x1t = np.ascontiguousarray(x1.T).astype(BF16)
    x2t = np.ascontiguousarray(x2.T).astype(BF16)

    def norm_adj_t(ei):
        ei = np.asarray(ei)
        A = np.zeros((N, N), np.float32)
        np.add.at(A, (ei[1], ei[0]), 1.0)
        deg = A.sum(1)
        A /= np.maximum(deg, 1.0)[:, None]
        return np.ascontiguousarray(A.T)  # MT[j, n]

    mt = [norm_adj_t(inputs["ei1"]), norm_adj_t(inputs["ei2"])]

    wm = np.zeros((14, 128, 128), np.float32)
    bs = np.zeros((8, 128, 1), np.float32)
    for l, s in enumerate(("1", "2")):
        wm[7 * l + WK] = inputs["Wk" + s]
        wm[7 * l + WQ] = inputs["Wq" + s]
        wm[7 * l + WV] = inputs["Wv" + s]
        wm[7 * l + WLS] = inputs["Wl" + s][:128] + inputs["Wl" + s][128:]
        wm[7 * l + WL1N] = -inputs["Wl" + s][128:]
        wm[7 * l + WRS] = inputs["Wr" + s][:128] + inputs["Wr" + s][128:]
        wm[7 * l + WR1N] = -inputs["Wr" + s][128:]
        bs[4 * l + BK, :, 0] = inputs["bk" + s]
        bs[4 * l + BQ, :, 0] = inputs["bq" + s]
        bs[4 * l + BV, :, 0] = inputs["bv" + s]
        bs[4 * l + BL, :, 0] = inputs["bl" + s]
    wm = wm.astype(BF16)

    in_maps = []
    for c in range(NCORES):
        sl = slice(c * SH, (c + 1) * SH)
        in_maps.append({
            "x1t": x1t,
            "x2t": x2t,
            "xown": np.stack([x1t[:, sl], x2t[:, sl]]),
            "mtc1": np.ascontiguousarray(
                mt[0][:, sl].astype(BF16).reshape(NJ, 128, SH)),
            "mtc2": np.ascontiguousarray(
                mt[1][:, sl].astype(BF16).reshape(NJ, 128, SH)),
            "wm": wm,
            "bs": bs,
        })
    return in_maps


def _assemble(results):
    full = np.empty((2 * N, 2 * N), np.float32)
    for c in range(NCORES):
        o = results[c]["out"]
        full[c * SH:(c + 1) * SH] = o[0]
        full[N + c * SH:N + (c + 1) * SH] = o[1]
    return full


def get_nc():
    if "nc" not in _cache:
        _cache["nc"] = _build_nc()
    return _cache["nc"]


def kernel(**inputs):
    nc = get_nc()
    in_maps = _host_prep(inputs)
    res = run_bass_kernel_spmd(nc, in_maps, core_ids=list(range(NCORES)))
    return _assemble(res.results)


# revision 34
# speedup vs baseline: 1.0459x; 1.0459x over previous
"""Trainium2 Bass kernel for nn_AttMatch (2-graph attention + SAGEConv GNN).

Self-contained: takes the full unsharded inputs of the reference problem,
shards across 8 NeuronCores internally, runs one SPMD NEFF, and gathers the
full [8192, 8192] sigmoid adjacency output.

Sharding (v5): query-sharded attention -- each core owns a 512-node slice of
EACH graph; keys/values cover the full 8192-row concatenated target set on
every core, so the softmax over targets is fully local (no AllReduce).

Collectives per layer (AllGather only, warmed up at start):
  A: out(g0)
  B (l=0): k|v|y own-slices for layer 1 (g0, computed with layer-1 weights
           right after h(g0)) merged with out(g1)    -> layer 1 needs no
  C (l=0): k|v|y own-slices (g1)                        projections at all
  B (l=1): h(g0) | out(g1)   (final-phase columns)
  C (l=1): h(g1)

SAGEConv is applied once per (layer, graph) via the algebraic merge
  h = relu( M @ (X@Wls - out@Wl1) + X_own@Wrs - out_own@Wr1 + bl ),
with M^T column shards loaded once (M is layer-invariant).  The g0 SAGE tail
is interleaved into attention(g1); the g1 tail is carried into the next
phase's PE stream.  The attention inner loop runs with a 2-pair lookahead so
the in-order PE queue never waits on exp or the epilogue.  Output in fp16.
"""

import numpy as np
import ml_dtypes

import concourse.bass as bass
import concourse.bacc as bacc
import concourse.tile as tile
import concourse.mybir as mybir
from concourse.bass_utils import run_bass_kernel_spmd

BF16 = ml_dtypes.bfloat16

N = 4096           # nodes per graph
D = 128            # feature dim (in == out == 128)
NCORES = 8
SH = N // NCORES   # 512-node own slice per graph per core
T = 2 * N          # concatenated target set
NTT = T // 128     # 64 target tiles
NPAIR = NTT // 2   # 32 pairs of target tiles (one exp per pair)
NJ = N // 128      # 32 source-node tiles per graph
INV_SCALE = 1.0 / np.sqrt(128.0)

F32 = mybir.dt.float32
BF = mybir.dt.bfloat16
FP16 = mybir.dt.float16

ADD = mybir.AluOpType.add
MULT = mybir.AluOpType.mult
MAX = mybir.AluOpType.max

# wm indices (per layer l: base = 7*l)
WK, WQ, WV, WLS, WL1N, WRS, WR1N = range(7)
# bias indices (per layer l: base = 4*l)
BK, BQ, BV, BL = range(4)

_cache = {}


def _build_nc():
    nc = bacc.Bacc("TRN2", target_bir_lowering=False, debug=False,
                   num_devices=NCORES)

    # ---- external I/O ----
    x1t = nc.dram_tensor("x1t", [D, N], BF, kind="ExternalInput")
    x2t = nc.dram_tensor("x2t", [D, N], BF, kind="ExternalInput")
    xgt_in = [x1t, x2t]
    xown_in = nc.dram_tensor("xown", [2, D, SH], BF, kind="ExternalInput")
    mtc_in = [nc.dram_tensor("mtc1", [NJ, 128, SH], BF, kind="ExternalInput"),
              nc.dram_tensor("mtc2", [NJ, 128, SH], BF, kind="ExternalInput")]
    wm_in = nc.dram_tensor("wm", [14, 128, 128], BF, kind="ExternalInput")
    bs_in = nc.dram_tensor("bs", [8, 128, 1], F32, kind="ExternalInput")
    out_ext = nc.dram_tensor("out", [2, SH, T], FP16, kind="ExternalOutput")

    # ---- internal DRAM for collectives ----
    rg = [list(range(NCORES))]
    ago_in = [nc.dram_tensor(f"ago_in_{l}", [D, SH], BF) for l in range(2)]
    ago_out = [nc.dram_tensor(f"ago_out_{l}", [NCORES, D, SH], BF,
                              addr_space="Shared") for l in range(2)]
    # AG-B payloads: l=0 -> [k0, v0, y0, out1]; l=1 -> [h0, out1]
    agb_n = [4, 2]
    agb_in = [nc.dram_tensor(f"agb_in_{l}", [agb_n[l], D, SH], BF)
              for l in range(2)]
    agb_out = [nc.dram_tensor(f"agb_out_{l}", [NCORES, agb_n[l], D, SH], BF,
                              addr_space="Shared") for l in range(2)]
    # AG-C payloads: l=0 -> [k1, v1, y1]; l=1 -> [h1]
    agc_n = [3, 1]
    agc_in = [nc.dram_tensor(f"agc_in_{l}", [agc_n[l], D, SH], BF)
              for l in range(2)]
    agc_out = [nc.dram_tensor(f"agc_out_{l}", [NCORES, agc_n[l], D, SH], BF,
                              addr_space="Shared") for l in range(2)]

    with tile.TileContext(nc) as tc:
        with (
            tc.tile_pool(name="const", bufs=1) as cpool,
            tc.tile_pool(name="xt", bufs=1) as xt_pool,
            tc.tile_pool(name="xo", bufs=2) as xo_pool,
            tc.tile_pool(name="kv", bufs=1) as kv_pool,
            tc.tile_pool(name="qt", bufs=1) as qt_pool,
            tc.tile_pool(name="es", bufs=5) as es_pool,
            tc.tile_pool(name="cs", bufs=2) as cs_pool,
            tc.tile_pool(name="st", bufs=2) as st_pool,
            tc.tile_pool(name="kvo", bufs=1) as kvo_pool,
            tc.tile_pool(name="y", bufs=1) as y_pool,
            tc.tile_pool(name="ub", bufs=1) as ub_pool,
            tc.tile_pool(name="of", bufs=1) as of_pool,
            tc.tile_pool(name="mt", bufs=16) as mt_pool,
            tc.tile_pool(name="z", bufs=3) as z_pool,
            tc.tile_pool(name="pa", bufs=2, space="PSUM") as pa_pool,
            tc.tile_pool(name="pb", bufs=1, space="PSUM") as pb_pool,
            tc.tile_pool(name="pc", bufs=2, space="PSUM") as pc_pool,
        ):
            # ---- constants ----
            wm = cpool.tile([128, 14 * 128], BF, name="wm_sb")
            nc.sync.dma_start(
                wm.rearrange("p (i f) -> p i f", i=14),
                wm_in.ap().rearrange("i p f -> p i f"))
            bs = cpool.tile([128, 8], F32, name="bs_sb")
            nc.sync.dma_start(
                bs.rearrange("p (i f) -> p i f", i=8),
                bs_in.ap().rearrange("i p f -> p i f"))
            ones_m1 = cpool.tile([128, 1], BF, name="ones_m1")
            nc.vector.memset(ones_m1[:], 1.0)
            ones_c = cpool.tile([1, 128], BF, name="ones_c")
            nc.vector.memset(ones_c[:], 1.0)

            def W(l, i):
                base = 7 * l + i
                return wm[:, 128 * base:128 * (base + 1)]

            def B(l, i):
                return bs[:, 4 * l + i:4 * l + i + 1]

            # warm up the collective rings FIRST (before any bulk load) so
            # the first-CC cold-start / inter-core skew is absorbed as early
            # as possible
            wc_in = nc.dram_tensor("wc_in", [128, 8], BF)
            wc_out = nc.dram_tensor("wc_out", [NCORES, 128, 8], BF,
                                    addr_space="Shared")
            nc.sync.dma_start(wc_in.ap(), wm[:, 0:8])
            nc.gpsimd.collective_compute(
                "AllGather", mybir.AluOpType.bypass, replica_groups=rg,
                ins=[wc_in.ap()], outs=[wc_out.ap()])

            # ---- generation-0 node features (split loads: compute starts
            # after the first chunk) ----
            xg = []
            for g in range(2):
                t = xt_pool.tile([D, N], BF, name=f"x{g}t_0", tag=f"xt{g}")
                for hh in range(2):
                    nc.sync.dma_start(t[:, hh * 2048:(hh + 1) * 2048],
                                      xgt_in[g][:, hh * 2048:(hh + 1) * 2048])
                xg.append(t)
            xo = []
            for g in range(2):
                t = xo_pool.tile([D, SH], BF, name=f"xown{g}_0", tag=f"xo{g}")
                nc.sync.dma_start(t[:], xown_in[g])
                xo.append(t)

            # M^T column shards: identical for both layers -- load once and
            # keep resident (16 tiles, never rewritten)
            mtcs = [[], []]
            for g in range(2):
                for jb in range(8):
                    mtc_t = mt_pool.tile([128, 4 * SH], BF, tag="mt",
                                         name=f"mtc_{g}_{jb}")
                    nc.sync.dma_start(
                        mtc_t.rearrange("p (j n) -> p j n", j=4),
                        mtc_in[g].ap()[4 * jb:4 * jb + 4]
                        .rearrange("j p n -> p j n"))
                    mtcs[g].append(mtc_t)

            hown_final = None
            xgt2 = [None, None]
            carry = []          # g1 SAGE tail of the previous layer

            for l in range(2):
                kt = kv_pool.tile([D, T], BF, name=f"kt_{l}", tag="kt")
                vn = kv_pool.tile([128, T], BF, name=f"vn_{l}", tag="vn")
                ybig = [y_pool.tile([128, N], BF, name=f"y_{l}_{g}",
                                    tag=f"y{g}") for g in range(2)]
                qt = [None, None]
                csa = [None, None]
                php = [None, None]
                outt = [None, None]
                psa = [None, None]
                hT = [None, None]
                xg_next = [None, None]
                pend = {0: [], 1: []}
                xg_l = xg
                xo_l = xo

                def proj_chunk(h, c, l=l, kt=kt, vn=vn, xg_l=xg_l):
                    """k^T and natural-layout v tiles, one 512-wide chunk of
                    target half h (layer 0 only; layer 1 gets k/v/y via AG)."""
                    ps = pc_pool.tile([128, 512], F32, tag="c",
                                      name=f"psk_{l}_{h}_{c}")
                    nc.tensor.matmul(ps[:], W(l, WK),
                                     xg_l[h][:, c * 512:(c + 1) * 512],
                                     start=True, stop=True)
                    nc.vector.tensor_scalar(
                        kt[:, h * N + c * 512:h * N + (c + 1) * 512],
                        ps[:], B(l, BK), None, ADD)
                    psv = pc_pool.tile([128, 512], F32, tag="c",
                                       name=f"psv_{l}_{h}_{c}")
                    for k in range(4):
                        t0 = c * 4 + k
                        nc.tensor.matmul(
                            psv[:, k * 128:(k + 1) * 128],
                            xg_l[h][:, t0 * 128:(t0 + 1) * 128],
                            W(l, WV), start=True, stop=True)
                    # v bias folded in post-softmax (sum(alpha)=1)
                    nc.vector.tensor_copy(
                        vn[:, h * N + c * 512:h * N + (c + 1) * 512],
                        psv[:])

                def gather_kvy(h, l=l, kt=kt, vn=vn, ybig=ybig):
                    """Layer 1: k/v/y for target half h arrive via AG-B/C of
                    layer 0 (computed there with layer-1 weights)."""
                    src = agb_out[0].ap() if h == 0 else agc_out[0].ap()
                    for i, dst in ((0, kt[:, h * N:(h + 1) * N]),
                                   (1, vn[:, h * N:(h + 1) * N]),
                                   (2, ybig[h][:])):
                        nc.gpsimd.dma_start(
                            dst.rearrange("p (c n) -> p c n", c=NCORES),
                            src[:, i].rearrange("c p n -> p c n"))

                def proj_q(g, l=l, qt=qt, xo_l=xo_l):
                    ps = pc_pool.tile([128, 512], F32, tag="c",
                                      name=f"psq_{l}_{g}")
                    nc.tensor.matmul(ps[:], W(l, WQ), xo_l[g][:],
                                     start=True, stop=True)
                    q = qt_pool.tile([D, SH], BF, name=f"qt_{l}_{g}",
                                     tag=f"qt{g}")
                    nc.vector.tensor_scalar(q[:], ps[:], B(l, BQ), None, ADD)
                    qt[g] = q

                def yfill(g, jb, l=l, ybig=ybig, xg_l=xg_l):
                    """Y = X @ Wls, natural layout, 4 node tiles (layer 0)."""
                    psy = pc_pool.tile([128, 512], F32, tag="c",
                                       name=f"psy_{l}_{g}_{jb}")
                    for k in range(4):
                        jt = jb * 4 + k
                        nc.tensor.matmul(psy[:, k * 128:(k + 1) * 128],
                                         xg_l[g][:, jt * 128:(jt + 1) * 128],
                                         W(l, WLS), start=True, stop=True)
                    nc.vector.tensor_copy(
                        ybig[g][:, jb * 512:(jb + 1) * 512], psy[:])

                def attn_scores(g, p, l=l, kt=kt, qt=qt, pend=pend):
                    ps_s = pa_pool.tile([128, 1024], F32, tag="ps_s",
                                        name=f"pss_{l}_{g}_{p}")
                    for i in range(2):
                        tt = 2 * p + i
                        nc.tensor.matmul(ps_s[:, i * 512:(i + 1) * 512],
                                         kt[:, tt * 128:(tt + 1) * 128],
                                         qt[g][:], start=True, stop=True)
                    es = es_pool.tile([128, 1024], BF, tag="es",
                                      name=f"es_{l}_{g}_{p}")
                    nc.scalar.activation(es[:], ps_s[:],
                                         mybir.ActivationFunctionType.Exp,
                                         scale=INV_SCALE)
                    pend[g].append((p, es))

                def attn_value(g, l=l, vn=vn, php=php, csa=csa, pend=pend):
                    p, es = pend[g].pop(0)
                    for i in range(2):
                        tt = 2 * p + i
                        nc.tensor.matmul(php[g][:],
                                         vn[:, tt * 128:(tt + 1) * 128],
                                         es[:, i * 512:(i + 1) * 512],
                                         start=(tt == 0), stop=(tt == NTT - 1))
                    if p == 0:
                        nc.vector.tensor_copy(csa[g][:], es[:])
                    else:
                        nc.vector.tensor_tensor(csa[g][:], csa[g][:], es[:],
                                                ADD)

                def epi(g, l=l, csa=csa, php=php, outt=outt):
                    """softmax epilogue: outT = php / colsum + bv.

                    g0 publishes via AG-A; g1 stages into the AG-B payload
                    (published together with the g0 tail)."""
                    csf = st_pool.tile([128, 512], BF, tag="csf",
                                       name=f"csf_{l}_{g}")
                    nc.vector.tensor_tensor(csf[:], csa[g][:, 0:512],
                                            csa[g][:, 512:1024], ADD)
                    pcs = pc_pool.tile([1, 512], F32, tag="c",
                                       name=f"pcs_{l}_{g}")
                    nc.tensor.matmul(pcs[:], ones_m1[:], csf[:],
                                     start=True, stop=True)
                    csb = st_pool.tile([1, 512], BF, tag="csb",
                                       name=f"csb_{l}_{g}")
                    nc.vector.tensor_copy(csb[:], pcs[:])
                    ps_rep = pc_pool.tile([128, 512], F32, tag="c",
                                          name=f"psrep_{l}_{g}")
                    nc.tensor.matmul(ps_rep[:], ones_c[:], csb[:],
                                     start=True, stop=True)
                    rr = st_pool.tile([128, 512], F32, tag="rr",
                                      name=f"rr_{l}_{g}")
                    nc.vector.reciprocal_approx_fast(rr[:], ps_rep[:])
                    pre = st_pool.tile([128, 512], F32, tag="pre",
                                       name=f"pre_{l}_{g}")
                    nc.vector.tensor_tensor(pre[:], php[g][:], rr[:], MULT)
                    ot = st_pool.tile([D, SH], BF, tag=f"ot{g}",
                                      name=f"outt_{l}_{g}")
                    nc.vector.tensor_scalar(ot[:], pre[:], B(l, BV), None,
                                            ADD)
                    outt[g] = ot
                    if g == 0:
                        nc.gpsimd.dma_start(ago_in[l][:], ot[:])
                        nc.gpsimd.collective_compute(
                            "AllGather", mybir.AluOpType.bypass,
                            replica_groups=rg,
                            ins=[ago_in[l][:]], outs=[ago_out[l][:]])
                    else:
                        nc.gpsimd.dma_start(
                            agb_in[l].ap()[agb_n[l] - 1], ot[:])

                def gather_of(g, l=l):
                    # post-collective gathers go on the gpsimd queue so the
                    # CC-semaphore wait never blocks the sync DMA queue
                    of = of_pool.tile([D, N], BF, tag="of",
                                      name=f"of_{l}_{g}")
                    src = (ago_out[l].ap() if g == 0
                           else agb_out[l].ap()[:, agb_n[l] - 1])
                    nc.gpsimd.dma_start(
                        of.rearrange("p (c n) -> p c n", c=NCORES),
                        src.rearrange("c p n -> p c n"))
                    return of

                def gather_x(g, l=l):
                    # layer 1 only: the full h (= final-phase F^T columns)
                    t = xt_pool.tile([D, N], BF, name=f"x{g}t_{l + 1}",
                                     tag=f"xt{g}")
                    src = (agb_out[l].ap()[:, 0] if g == 0
                           else agc_out[l].ap()[:, 0])
                    nc.gpsimd.dma_start(
                        t.rearrange("p (c n) -> p c n", c=NCORES),
                        src.rearrange("c p n -> p c n"))
                    return t

                def ufill(g, ofl, ub, jb, l=l):
                    """U' = out @ (-Wl1), natural layout, 4 node tiles."""
                    psu = pc_pool.tile([128, 512], F32, tag="c",
                                       name=f"psu_{l}_{g}_{jb}")
                    for k in range(4):
                        jt = jb * 4 + k
                        nc.tensor.matmul(psu[:, k * 128:(k + 1) * 128],
                                         ofl[0][:, jt * 128:(jt + 1) * 128],
                                         W(l, WL1N), start=True, stop=True)
                    nc.vector.tensor_copy(ub[:, jb * 512:(jb + 1) * 512],
                                          psu[:])

                def vsub(g, ub, jb, ybig=ybig):
                    # V = Y + U', one 512-wide chunk (in place over ybig) so
                    # each M group unblocks as soon as its chunk is ready
                    sl = slice(jb * 512, (jb + 1) * 512)
                    nc.vector.tensor_tensor(ybig[g][:, sl], ybig[g][:, sl],
                                            ub[:, sl], ADD)

                def mfill(g, jb, l=l, psa=psa, ybig=ybig):
                    if psa[g] is None:
                        psa[g] = pb_pool.tile([128, 512], F32, tag="psa",
                                              name=f"psa_{l}_{g}")
                    for k in range(4):
                        jt = jb * 4 + k
                        nc.tensor.matmul(psa[g][:],
                                         ybig[g][:, jt * 128:(jt + 1) * 128],
                                         mtcs[g][jb][:, k * SH:(k + 1) * SH],
                                         start=(jt == 0), stop=False)

                def ownparts(g, stage, l=l, hT=hT):
                    """Layer 0 tails: produce layer-1 k/v/y for the own node
                    slice (from h) and stage them into the AG payload."""
                    h = hT[g]
                    psk = pc_pool.tile([128, 512], F32, tag="c",
                                       name=f"psok_{l}_{g}")
                    nc.tensor.matmul(psk[:], W(1, WK), h[:],
                                     start=True, stop=True)
                    ko = kvo_pool.tile([D, 3 * SH], BF, tag=f"ko{g}",
                                       name=f"ko_{l}_{g}")
                    nc.vector.tensor_scalar(ko[:, 0:512], psk[:], B(1, BK),
                                            None, ADD)
                    psv = pc_pool.tile([128, 512], F32, tag="c",
                                       name=f"psov_{l}_{g}")
                    for k in range(4):
                        nc.tensor.matmul(psv[:, k * 128:(k + 1) * 128],
                                         h[:, k * 128:(k + 1) * 128],
                                         W(1, WV), start=True, stop=True)
                    nc.vector.tensor_copy(ko[:, 512:1024], psv[:])
                    psy = pc_pool.tile([128, 512], F32, tag="c",
                                       name=f"psoy_{l}_{g}")
                    for k in range(4):
                        nc.tensor.matmul(psy[:, k * 128:(k + 1) * 128],
                                         h[:, k * 128:(k + 1) * 128],
                                         W(1, WLS), start=True, stop=True)
                    nc.vector.tensor_copy(ko[:, 1024:1536], psy[:])
                    nc.gpsimd.dma_start(
                        stage.ap()[0:3].rearrange("i p n -> p i n"),
                        ko.rearrange("p (i n) -> p i n", i=3))

                def tail_end(g, l=l, psa=psa, outt=outt, hT=hT, xo_l=xo_l,
                             ownparts=ownparts):
                    """Wr terms + bias(+relu); publish AG-B (g0) / AG-C (g1)."""
                    nc.tensor.matmul(psa[g][:], W(l, WRS), xo_l[g][:],
                                     start=False, stop=False)
                    nc.tensor.matmul(psa[g][:], W(l, WR1N), outt[g][:],
                                     start=False, stop=True)
                    h = xo_pool.tile([D, SH], BF, name=f"hown_{l}_{g}",
                                     tag=f"xo{g}")
                    if l == 0:
                        nc.vector.tensor_scalar(h[:], psa[g][:], B(l, BL),
                                                0.0, ADD, MAX)
                    else:
                        nc.vector.tensor_scalar(h[:], psa[g][:], B(l, BL),
                                                None, ADD)
                    hT[g] = h
                    stage = agb_in[l] if g == 0 else agc_in[l]
                    if l == 0:
                        ownparts(g, stage)
                    else:
                        nc.gpsimd.dma_start(stage.ap()[0], h[:])
                    src_t = agb_in[l] if g == 0 else agc_in[l]
                    dst_t = agb_out[l] if g == 0 else agc_out[l]
                    nc.gpsimd.collective_compute(
                        "AllGather", mybir.AluOpType.bypass,
                        replica_groups=rg,
                        ins=[src_t[:]], outs=[dst_t[:]])

                # ================= layer emission =================
                php[0] = pb_pool.tile([128, 512], F32, tag="php",
                                      name=f"php_{l}_0")
                csa[0] = cs_pool.tile([128, 1024], BF, tag="csa",
                                      name=f"csa_{l}_0")
                csa[1] = cs_pool.tile([128, 1024], BF, tag="csa",
                                      name=f"csa_{l}_1")

                if l == 0:
                    for c in range(8):
                        proj_chunk(0, c)
                    proj_q(0)
                    for jb in range(8):
                        yfill(0, jb)
                    # second-half projections interleave into pairs 6..13
                    ins2 = []
                    for c in range(8):
                        ins2.append(lambda c=c: proj_chunk(1, c))
                        ins2.append(lambda c=c: yfill(1, c))
                else:
                    gather_kvy(0)
                    proj_q(0)
                    ins2 = []

                # attention(g0): previous layer's g1 SAGE tail interleaved
                # (3 items/pair -- its deps precede this layer's start), plus
                # the half-1 projections for l=0
                for p in range(NPAIR):
                    attn_scores(0, p)
                    if len(pend[0]) > 2:
                        attn_value(0)
                    for _ in range(4):
                        if carry:
                            carry.pop(0)()
                    if p >= 6:
                        for _ in range(2):
                            if ins2:
                                ins2.pop(0)()
                    if p == 15:
                        proj_q(1)   # xo[1]: external (l0) / carried h (l1)
                        if l == 1:
                            gather_kvy(1)
                while pend[0]:
                    attn_value(0)
                epi(0)

                # attention(g1) with the g0 SAGE tail interleaved
                php[1] = pb_pool.tile([128, 512], F32, tag="php",
                                      name=f"php_{l}_1")
                fill = []
                of0 = [None]
                ub0 = ub_pool.tile([128, N], BF, tag="ub", name=f"ub_{l}_0")

                def mk_uf(jb):
                    return lambda: ufill(0, of0, ub0, jb)

                def mk_sub(jb):
                    return lambda: vsub(0, ub0, jb)

                def mk_mf(jb):
                    return lambda: mfill(0, jb)

                for jb in range(8):
                    fill.append(mk_uf(jb))
                    fill.append(mk_sub(jb))
                for jb in range(8):
                    fill.append(mk_mf(jb))

                # AG-A absorbs the residual inter-core skew on layer 0, so
                # its result lands much later there -- start the g0-tail
                # fillers later to keep the in-order PE queue from stalling
                f_start = 26 if l == 0 else 14
                for p in range(NPAIR):
                    attn_scores(1, p)
                    if len(pend[1]) > 2:
                        attn_value(1)
                    if p == f_start - 2:
                        of0[0] = gather_of(0)
                    if p >= f_start:
                        for _ in range(3 if l == 0 else 2):
                            if fill:
                                fill.pop(0)()
                while pend[1]:
                    attn_value(1)
                epi(1)
                while fill:
                    fill.pop(0)()
                tail_end(0)                       # issues AG-B
                if l == 1:
                    xg_next[0] = gather_x(0)
                of1 = [gather_of(1)]

                # g1 SAGE tail -> carried into the next phase's PE stream.
                # NOTE: bind every per-layer name via default args -- the
                # loop body is one scope and rebinds them next iteration.
                ub1 = ub_pool.tile([128, N], BF, tag="ub", name=f"ub_{l}_1")

                def mk_uf1(jb, f=ufill, o=of1, u=ub1):
                    return lambda: f(1, o, u, jb)

                def mk_sub1(jb, f=vsub, u=ub1):
                    return lambda: f(1, u, jb)

                def mk_mf1(jb, f=mfill):
                    return lambda: f(1, jb)

                carry = []
                for jb in range(8):
                    carry.append(mk_uf1(jb))
                    carry.append(mk_sub1(jb))
                for jb in range(8):
                    carry.append(mk_mf1(jb))
                carry.append(lambda f=tail_end: f(1))
                if l == 1:
                    carry.append(
                        lambda xn=xg_next, f=gather_x: xn.__setitem__(
                            1, f(1)))

                if l == 0:
                    xo = hT
                else:
                    hown_final = hT
                    xgt2 = xg_next

            # ---- final adjacency: sigmoid(F @ F^T), own 1024 rows ----
            def final_block(ch, grow, rt, cp):
                lhs = hown_final[grow][:, rt * 128:(rt + 1) * 128]
                ps_z = pa_pool.tile([128, 1024], F32, tag="ps_s",
                                    name=f"psz_{ch}_{grow}_{rt}_{cp}")
                for i in range(2):
                    c0 = cp * 1024 + i * 512
                    nc.tensor.matmul(
                        ps_z[:, i * 512:(i + 1) * 512], lhs,
                        xgt2[ch][:, c0:c0 + 512],
                        start=True, stop=True)
                z = z_pool.tile([128, 1024], FP16, tag="z",
                                name=f"z_{ch}_{grow}_{rt}_{cp}")
                nc.scalar.activation(
                    z[:], ps_z[:], mybir.ActivationFunctionType.Sigmoid)
                nc.sync.dma_start(
                    out_ext[grow, rt * 128:(rt + 1) * 128,
                            ch * N + cp * 1024:ch * N + (cp + 1) * 1024],
                    z[:])

            def final_step(ch, grow, rt, cp):
                final_block(ch, grow, rt, cp)
                for _ in range(4):
                    if carry:
                        carry.pop(0)()

            blocks = [(ch, grow, rt, cp) for ch in range(2)
                      for grow in range(2) for rt in range(4)
                      for cp in range(4)]
            for blk in blocks:
                final_step(*blk)

    nc.compile()
    return nc


def _host_prep(inputs):
    """Build per-core input maps from the full problem inputs."""
    x1 = np.asarray(inputs["x1"], np.float32)
    x2 = np.asarray(inputs["x2"], np.float32)
    x1t = np.ascontiguousarray(x1.T).astype(BF16)
    x2t = np.ascontiguousarray(x2.T).astype(BF16)

    def norm_adj_t(ei):
        ei = np.asarray(ei)
        A = np.zeros((N, N), np.float32)
        np.add.at(A, (ei[1], ei[0]), 1.0)
        deg = A.sum(1)
        A /= np.maximum(deg, 1.0)[:, None]
        return np.ascontiguousarray(A.T)  # MT[j, n]

    mt = [norm_adj_t(inputs["ei1"]), norm_adj_t(inputs["ei2"])]

    wm = np.zeros((14, 128, 128), np.float32)
    bs = np.zeros((8, 128, 1), np.float32)
    for l, s in enumerate(("1", "2")):
        wm[7 * l + WK] = inputs["Wk" + s]
        wm[7 * l + WQ] = inputs["Wq" + s]
        wm[7 * l + WV] = inputs["Wv" + s]
        wm[7 * l + WLS] = inputs["Wl" + s][:128] + inputs["Wl" + s][128:]
        wm[7 * l + WL1N] = -inputs["Wl" + s][128:]
        wm[7 * l + WRS] = inputs["Wr" + s][:128] + inputs["Wr" + s][128:]
        wm[7 * l + WR1N] = -inputs["Wr" + s][128:]
        bs[4 * l + BK, :, 0] = inputs["bk" + s]
        bs[4 * l + BQ, :, 0] = inputs["bq" + s]
        bs[4 * l + BV, :, 0] = inputs["bv" + s]
        bs[4 * l + BL, :, 0] = inputs["bl" + s]
    wm = wm.astype(BF16)

    in_maps = []
    for c in range(NCORES):
        sl = slice(c * SH, (c + 1) * SH)
        in_maps.append({
            "x1t": x1t,
            "x2t": x2t,
            "xown": np.stack([x1t[:, sl], x2t[:, sl]]),
            "mtc1": np.ascontiguousarray(
                mt[0][:, sl].astype(BF16).reshape(NJ, 128, SH)),
            "mtc2": np.ascontiguousarray(
                mt[1][:, sl].astype(BF16).reshape(NJ, 128, SH)),
            "wm": wm,
            "bs": bs,
        })
    return in_maps


def _assemble(results):
    full = np.empty((2 * N, 2 * N), np.float32)
    for c in range(NCORES):
        o = results[c]["out"]
        full[c * SH:(c + 1) * SH] = o[0]
        full[N + c * SH:N + (c + 1) * SH] = o[1]
    return full


def get_nc():
    if "nc" not in _cache:
        _cache["nc"] = _build_nc()
    return _cache["nc"]


def kernel(**inputs):
    nc = get_nc()
    in_maps = _host_prep(inputs)
    res = run_bass_kernel_spmd(nc, in_maps, core_ids=list(range(NCORES)))
    return _assemble(res.results)


# revision 37
# speedup vs baseline: 1.0521x; 1.0059x over previous
"""Trainium2 Bass kernel for nn_AttMatch (2-graph attention + SAGEConv GNN).

Self-contained: takes the full unsharded inputs of the reference problem,
shards across 8 NeuronCores internally, runs one SPMD NEFF, and gathers the
full [8192, 8192] sigmoid adjacency output.

Sharding (v5): query-sharded attention -- each core owns a 512-node slice of
EACH graph; keys/values cover the full 8192-row concatenated target set on
every core, so the softmax over targets is fully local (no AllReduce).

Collectives per layer (AllGather only, warmed up at start):
  A: out(g0)
  B (l=0): k|v|y own-slices for layer 1 (g0, computed with layer-1 weights
           right after h(g0)) merged with out(g1)    -> layer 1 needs no
  C (l=0): k|v|y own-slices (g1)                        projections at all
  B (l=1): h(g0) | out(g1)   (final-phase columns)
  C (l=1): h(g1)

SAGEConv is applied once per (layer, graph) via the algebraic merge
  h = relu( M @ (X@Wls - out@Wl1) + X_own@Wrs - out_own@Wr1 + bl ),
with M^T column shards loaded once (M is layer-invariant).  The g0 SAGE tail
is interleaved into attention(g1); the g1 tail is carried into the next
phase's PE stream.  The attention inner loop runs with a 2-pair lookahead so
the in-order PE queue never waits on exp or the epilogue.  Output in fp16.
"""

import numpy as np
import ml_dtypes

import concourse.bass as bass
import concourse.bacc as bacc
import concourse.tile as tile
import concourse.mybir as mybir
from concourse.bass_utils import run_bass_kernel_spmd

BF16 = ml_dtypes.bfloat16

N = 4096           # nodes per graph
D = 128            # feature dim (in == out == 128)
NCORES = 8
SH = N // NCORES   # 512-node own slice per graph per core
T = 2 * N          # concatenated target set
NTT = T // 128     # 64 target tiles
NPAIR = NTT // 2   # 32 pairs of target tiles (one exp per pair)
NJ = N // 128      # 32 source-node tiles per graph
INV_SCALE = 1.0 / np.sqrt(128.0)

F32 = mybir.dt.float32
BF = mybir.dt.bfloat16
FP16 = mybir.dt.float16

ADD = mybir.AluOpType.add
MULT = mybir.AluOpType.mult
MAX = mybir.AluOpType.max

# wm indices (per layer l: base = 7*l)
WK, WQ, WV, WLS, WL1N, WRS, WR1N = range(7)
# bias indices (per layer l: base = 4*l)
BK, BQ, BV, BL = range(4)

_cache = {}


def _build_nc():
    nc = bacc.Bacc("TRN2", target_bir_lowering=False, debug=False,
                   num_devices=NCORES)

    # ---- external I/O ----
    x1t = nc.dram_tensor("x1t", [D, N], BF, kind="ExternalInput")
    x2t = nc.dram_tensor("x2t", [D, N], BF, kind="ExternalInput")
    xgt_in = [x1t, x2t]
    xown_in = nc.dram_tensor("xown", [2, D, SH], BF, kind="ExternalInput")
    mtc_in = [nc.dram_tensor("mtc1", [NJ, 128, SH], BF, kind="ExternalInput"),
              nc.dram_tensor("mtc2", [NJ, 128, SH], BF, kind="ExternalInput")]
    wm_in = nc.dram_tensor("wm", [14, 128, 128], BF, kind="ExternalInput")
    bs_in = nc.dram_tensor("bs", [8, 128, 1], F32, kind="ExternalInput")
    out_ext = nc.dram_tensor("out", [2, SH, T], FP16, kind="ExternalOutput")

    # ---- internal DRAM for collectives ----
    rg = [list(range(NCORES))]
    ago_in = [nc.dram_tensor(f"ago_in_{l}", [D, SH], BF) for l in range(2)]
    ago_out = [nc.dram_tensor(f"ago_out_{l}", [NCORES, D, SH], BF,
                              addr_space="Shared") for l in range(2)]
    # AG-B payloads: l=0 -> [k0, v0, y0, out1]; l=1 -> [h0, out1]
    agb_n = [4, 2]
    agb_in = [nc.dram_tensor(f"agb_in_{l}", [agb_n[l], D, SH], BF)
              for l in range(2)]
    agb_out = [nc.dram_tensor(f"agb_out_{l}", [NCORES, agb_n[l], D, SH], BF,
                              addr_space="Shared") for l in range(2)]
    # AG-C payloads: l=0 -> [k1, v1, y1]; l=1 -> [h1]
    agc_n = [3, 1]
    agc_in = [nc.dram_tensor(f"agc_in_{l}", [agc_n[l], D, SH], BF)
              for l in range(2)]
    agc_out = [nc.dram_tensor(f"agc_out_{l}", [NCORES, agc_n[l], D, SH], BF,
                              addr_space="Shared") for l in range(2)]

    with tile.TileContext(nc) as tc:
        with (
            tc.tile_pool(name="const", bufs=1) as cpool,
            tc.tile_pool(name="xt", bufs=1) as xt_pool,
            tc.tile_pool(name="xo", bufs=2) as xo_pool,
            tc.tile_pool(name="kv", bufs=1) as kv_pool,
            tc.tile_pool(name="qt", bufs=1) as qt_pool,
            tc.tile_pool(name="es", bufs=5) as es_pool,
            tc.tile_pool(name="cs", bufs=2) as cs_pool,
            tc.tile_pool(name="st", bufs=2) as st_pool,
            tc.tile_pool(name="kvo", bufs=1) as kvo_pool,
            tc.tile_pool(name="y", bufs=1) as y_pool,
            tc.tile_pool(name="ub", bufs=1) as ub_pool,
            tc.tile_pool(name="of", bufs=1) as of_pool,
            tc.tile_pool(name="mt", bufs=16) as mt_pool,
            tc.tile_pool(name="z", bufs=3) as z_pool,
            tc.tile_pool(name="pa", bufs=2, space="PSUM") as pa_pool,
            tc.tile_pool(name="pb", bufs=1, space="PSUM") as pb_pool,
            tc.tile_pool(name="pc", bufs=2, space="PSUM") as pc_pool,
        ):
            # ---- constants ----
            wm = cpool.tile([128, 14 * 128], BF, name="wm_sb")
            nc.sync.dma_start(
                wm.rearrange("p (i f) -> p i f", i=14),
                wm_in.ap().rearrange("i p f -> p i f"))
            bs = cpool.tile([128, 8], F32, name="bs_sb")
            nc.sync.dma_start(
                bs.rearrange("p (i f) -> p i f", i=8),
                bs_in.ap().rearrange("i p f -> p i f"))
            ones_m1 = cpool.tile([128, 1], BF, name="ones_m1")
            nc.vector.memset(ones_m1[:], 1.0)
            ones_c = cpool.tile([1, 128], BF, name="ones_c")
            nc.vector.memset(ones_c[:], 1.0)

            def W(l, i):
                base = 7 * l + i
                return wm[:, 128 * base:128 * (base + 1)]

            def B(l, i):
                return bs[:, 4 * l + i:4 * l + i + 1]

            # warm up the collective rings FIRST (before any bulk load) so
            # the first-CC cold-start / inter-core skew is absorbed as early
            # as possible
            wc_in = nc.dram_tensor("wc_in", [128, 8], BF)
            wc_out = nc.dram_tensor("wc_out", [NCORES, 128, 8], BF,
                                    addr_space="Shared")
            nc.sync.dma_start(wc_in.ap(), wm[:, 0:8])
            nc.gpsimd.collective_compute(
                "AllGather", mybir.AluOpType.bypass, replica_groups=rg,
                ins=[wc_in.ap()], outs=[wc_out.ap()])

            # ---- generation-0 node features (split loads: compute starts
            # after the first chunk) ----
            xg = []
            for g in range(2):
                t = xt_pool.tile([D, N], BF, name=f"x{g}t_0", tag=f"xt{g}")
                for hh in range(2):
                    nc.sync.dma_start(t[:, hh * 2048:(hh + 1) * 2048],
                                      xgt_in[g][:, hh * 2048:(hh + 1) * 2048])
                xg.append(t)
            xo = []
            for g in range(2):
                t = xo_pool.tile([D, SH], BF, name=f"xown{g}_0", tag=f"xo{g}")
                nc.sync.dma_start(t[:], xown_in[g])
                xo.append(t)

            # M^T column shards: identical for both layers -- load once and
            # keep resident (16 tiles, never rewritten)
            mtcs = [[], []]
            for g in range(2):
                for jb in range(8):
                    mtc_t = mt_pool.tile([128, 4 * SH], BF, tag="mt",
                                         name=f"mtc_{g}_{jb}")
                    nc.sync.dma_start(
                        mtc_t.rearrange("p (j n) -> p j n", j=4),
                        mtc_in[g].ap()[4 * jb:4 * jb + 4]
                        .rearrange("j p n -> p j n"))
                    mtcs[g].append(mtc_t)

            hown_final = None
            xgt2 = [None, None]
            carry = []          # g1 SAGE tail of the previous layer

            for l in range(2):
                kt = kv_pool.tile([D, T], BF, name=f"kt_{l}", tag="kt")
                vn = kv_pool.tile([128, T], BF, name=f"vn_{l}", tag="vn")
                ybig = [y_pool.tile([128, N], BF, name=f"y_{l}_{g}",
                                    tag=f"y{g}") for g in range(2)]
                qt = [None, None]
                csa = [None, None]
                php = [None, None]
                outt = [None, None]
                psa = [None, None]
                hT = [None, None]
                xg_next = [None, None]
                pend = {0: [], 1: []}
                xg_l = xg
                xo_l = xo

                def proj_chunk(h, c, l=l, kt=kt, vn=vn, xg_l=xg_l):
                    """k^T and natural-layout v tiles, one 512-wide chunk of
                    target half h (layer 0 only; layer 1 gets k/v/y via AG)."""
                    ps = pc_pool.tile([128, 512], F32, tag="c",
                                      name=f"psk_{l}_{h}_{c}")
                    nc.tensor.matmul(ps[:], W(l, WK),
                                     xg_l[h][:, c * 512:(c + 1) * 512],
                                     start=True, stop=True)
                    nc.vector.tensor_scalar(
                        kt[:, h * N + c * 512:h * N + (c + 1) * 512],
                        ps[:], B(l, BK), None, ADD)
                    psv = pc_pool.tile([128, 512], F32, tag="c",
                                       name=f"psv_{l}_{h}_{c}")
                    for k in range(4):
                        t0 = c * 4 + k
                        nc.tensor.matmul(
                            psv[:, k * 128:(k + 1) * 128],
                            xg_l[h][:, t0 * 128:(t0 + 1) * 128],
                            W(l, WV), start=True, stop=True)
                    # v bias folded in post-softmax (sum(alpha)=1)
                    nc.vector.tensor_copy(
                        vn[:, h * N + c * 512:h * N + (c + 1) * 512],
                        psv[:])

                def gather_kvy(h, l=l, kt=kt, vn=vn, ybig=ybig):
                    """Layer 1: k/v/y for target half h arrive via AG-B/C of
                    layer 0 (computed there with layer-1 weights).  k and v
                    are gathered in core-block halves so the first attention
                    pairs unblock before the full gather lands."""
                    src = agb_out[0].ap() if h == 0 else agc_out[0].ap()
                    for i, dst in ((0, kt[:, h * N:(h + 1) * N]),
                                   (1, vn[:, h * N:(h + 1) * N])):
                        for q in range(2):
                            cs_ = slice(q * 4, (q + 1) * 4)
                            nc.gpsimd.dma_start(
                                dst[:, q * 2048:(q + 1) * 2048]
                                .rearrange("p (c n) -> p c n", c=4),
                                src[cs_, i].rearrange("c p n -> p c n"))
                    nc.gpsimd.dma_start(
                        ybig[h].rearrange("p (c n) -> p c n", c=NCORES),
                        src[:, 2].rearrange("c p n -> p c n"))

                def proj_q(g, l=l, qt=qt, xo_l=xo_l):
                    ps = pc_pool.tile([128, 512], F32, tag="c",
                                      name=f"psq_{l}_{g}")
                    nc.tensor.matmul(ps[:], W(l, WQ), xo_l[g][:],
                                     start=True, stop=True)
                    q = qt_pool.tile([D, SH], BF, name=f"qt_{l}_{g}",
                                     tag=f"qt{g}")
                    nc.vector.tensor_scalar(q[:], ps[:], B(l, BQ), None, ADD)
                    qt[g] = q

                def yfill(g, jb, l=l, ybig=ybig, xg_l=xg_l):
                    """Y = X @ Wls, natural layout, 4 node tiles (layer 0)."""
                    psy = pc_pool.tile([128, 512], F32, tag="c",
                                       name=f"psy_{l}_{g}_{jb}")
                    for k in range(4):
                        jt = jb * 4 + k
                        nc.tensor.matmul(psy[:, k * 128:(k + 1) * 128],
                                         xg_l[g][:, jt * 128:(jt + 1) * 128],
                                         W(l, WLS), start=True, stop=True)
                    nc.vector.tensor_copy(
                        ybig[g][:, jb * 512:(jb + 1) * 512], psy[:])

                def attn_scores(g, p, l=l, kt=kt, qt=qt, pend=pend):
                    ps_s = pa_pool.tile([128, 1024], F32, tag="ps_s",
                                        name=f"pss_{l}_{g}_{p}")
                    for i in range(2):
                        tt = 2 * p + i
                        nc.tensor.matmul(ps_s[:, i * 512:(i + 1) * 512],
                                         kt[:, tt * 128:(tt + 1) * 128],
                                         qt[g][:], start=True, stop=True)
                    es = es_pool.tile([128, 1024], BF, tag="es",
                                      name=f"es_{l}_{g}_{p}")
                    nc.scalar.activation(es[:], ps_s[:],
                                         mybir.ActivationFunctionType.Exp,
                                         scale=INV_SCALE)
                    pend[g].append((p, es))

                def attn_value(g, l=l, vn=vn, php=php, csa=csa, pend=pend):
                    p, es = pend[g].pop(0)
                    for i in range(2):
                        tt = 2 * p + i
                        nc.tensor.matmul(php[g][:],
                                         vn[:, tt * 128:(tt + 1) * 128],
                                         es[:, i * 512:(i + 1) * 512],
                                         start=(tt == 0), stop=(tt == NTT - 1))
                    if p == 0:
                        nc.vector.tensor_copy(csa[g][:], es[:])
                    else:
                        nc.vector.tensor_tensor(csa[g][:], csa[g][:], es[:],
                                                ADD)

                def epi(g, l=l, csa=csa, php=php, outt=outt):
                    """softmax epilogue: outT = php / colsum + bv.

                    g0 publishes via AG-A; g1 stages into the AG-B payload
                    (published together with the g0 tail)."""
                    csf = st_pool.tile([128, 512], BF, tag="csf",
                                       name=f"csf_{l}_{g}")
                    nc.vector.tensor_tensor(csf[:], csa[g][:, 0:512],
                                            csa[g][:, 512:1024], ADD)
                    pcs = pc_pool.tile([1, 512], F32, tag="c",
                                       name=f"pcs_{l}_{g}")
                    nc.tensor.matmul(pcs[:], ones_m1[:], csf[:],
                                     start=True, stop=True)
                    csb = st_pool.tile([1, 512], BF, tag="csb",
                                       name=f"csb_{l}_{g}")
                    nc.vector.tensor_copy(csb[:], pcs[:])
                    ps_rep = pc_pool.tile([128, 512], F32, tag="c",
                                          name=f"psrep_{l}_{g}")
                    nc.tensor.matmul(ps_rep[:], ones_c[:], csb[:],
                                     start=True, stop=True)
                    rr = st_pool.tile([128, 512], F32, tag="rr",
                                      name=f"rr_{l}_{g}")
                    nc.vector.reciprocal_approx_fast(rr[:], ps_rep[:])
                    pre = st_pool.tile([128, 512], F32, tag="pre",
                                       name=f"pre_{l}_{g}")
                    nc.vector.tensor_tensor(pre[:], php[g][:], rr[:], MULT)
                    ot = st_pool.tile([D, SH], BF, tag=f"ot{g}",
                                      name=f"outt_{l}_{g}")
                    nc.vector.tensor_scalar(ot[:], pre[:], B(l, BV), None,
                                            ADD)
                    outt[g] = ot
                    if g == 0:
                        nc.gpsimd.dma_start(ago_in[l][:], ot[:])
                        nc.gpsimd.collective_compute(
                            "AllGather", mybir.AluOpType.bypass,
                            replica_groups=rg,
                            ins=[ago_in[l][:]], outs=[ago_out[l][:]])
                    else:
                        nc.gpsimd.dma_start(
                            agb_in[l].ap()[agb_n[l] - 1], ot[:])

                def gather_of(g, l=l):
                    # post-collective gathers go on the gpsimd queue so the
                    # CC-semaphore wait never blocks the sync DMA queue
                    of = of_pool.tile([D, N], BF, tag="of",
                                      name=f"of_{l}_{g}")
                    src = (ago_out[l].ap() if g == 0
                           else agb_out[l].ap()[:, agb_n[l] - 1])
                    nc.gpsimd.dma_start(
                        of.rearrange("p (c n) -> p c n", c=NCORES),
                        src.rearrange("c p n -> p c n"))
                    return of

                def gather_x(g, l=l):
                    # layer 1 only: the full h (= final-phase F^T columns)
                    t = xt_pool.tile([D, N], BF, name=f"x{g}t_{l + 1}",
                                     tag=f"xt{g}")
                    src = (agb_out[l].ap()[:, 0] if g == 0
                           else agc_out[l].ap()[:, 0])
                    nc.gpsimd.dma_start(
                        t.rearrange("p (c n) -> p c n", c=NCORES),
                        src.rearrange("c p n -> p c n"))
                    return t

                def ufill(g, ofl, ub, jb, l=l):
                    """U' = out @ (-Wl1), natural layout, 4 node tiles."""
                    psu = pc_pool.tile([128, 512], F32, tag="c",
                                       name=f"psu_{l}_{g}_{jb}")
                    for k in range(4):
                        jt = jb * 4 + k
                        nc.tensor.matmul(psu[:, k * 128:(k + 1) * 128],
                                         ofl[0][:, jt * 128:(jt + 1) * 128],
                                         W(l, WL1N), start=True, stop=True)
                    nc.vector.tensor_copy(ub[:, jb * 512:(jb + 1) * 512],
                                          psu[:])

                def vsub(g, ub, jb, ybig=ybig):
                    # V = Y + U', one 512-wide chunk (in place over ybig) so
                    # each M group unblocks as soon as its chunk is ready
                    sl = slice(jb * 512, (jb + 1) * 512)
                    nc.vector.tensor_tensor(ybig[g][:, sl], ybig[g][:, sl],
                                            ub[:, sl], ADD)

                def mfill(g, jb, l=l, psa=psa, ybig=ybig):
                    if psa[g] is None:
                        psa[g] = pb_pool.tile([128, 512], F32, tag="psa",
                                              name=f"psa_{l}_{g}")
                    for k in range(4):
                        jt = jb * 4 + k
                        nc.tensor.matmul(psa[g][:],
                                         ybig[g][:, jt * 128:(jt + 1) * 128],
                                         mtcs[g][jb][:, k * SH:(k + 1) * SH],
                                         start=(jt == 0), stop=False)

                def ownparts(g, stage, l=l, hT=hT):
                    """Layer 0 tails: produce layer-1 k/v/y for the own node
                    slice (from h) and stage them into the AG payload."""
                    h = hT[g]
                    psk = pc_pool.tile([128, 512], F32, tag="c",
                                       name=f"psok_{l}_{g}")
                    nc.tensor.matmul(psk[:], W(1, WK), h[:],
                                     start=True, stop=True)
                    ko = kvo_pool.tile([D, 3 * SH], BF, tag=f"ko{g}",
                                       name=f"ko_{l}_{g}")
                    nc.vector.tensor_scalar(ko[:, 0:512], psk[:], B(1, BK),
                                            None, ADD)
                    psv = pc_pool.tile([128, 512], F32, tag="c",
                                       name=f"psov_{l}_{g}")
                    for k in range(4):
                        nc.tensor.matmul(psv[:, k * 128:(k + 1) * 128],
                                         h[:, k * 128:(k + 1) * 128],
                                         W(1, WV), start=True, stop=True)
                    nc.vector.tensor_copy(ko[:, 512:1024], psv[:])
                    psy = pc_pool.tile([128, 512], F32, tag="c",
                                       name=f"psoy_{l}_{g}")
                    for k in range(4):
                        nc.tensor.matmul(psy[:, k * 128:(k + 1) * 128],
                                         h[:, k * 128:(k + 1) * 128],
                                         W(1, WLS), start=True, stop=True)
                    nc.vector.tensor_copy(ko[:, 1024:1536], psy[:])
                    nc.gpsimd.dma_start(
                        stage.ap()[0:3].rearrange("i p n -> p i n"),
                        ko.rearrange("p (i n) -> p i n", i=3))

                def tail_end(g, l=l, psa=psa, outt=outt, hT=hT, xo_l=xo_l,
                             ownparts=ownparts):
                    """Wr terms + bias(+relu); publish AG-B (g0) / AG-C (g1)."""
                    nc.tensor.matmul(psa[g][:], W(l, WRS), xo_l[g][:],
                                     start=False, stop=False)
                    nc.tensor.matmul(psa[g][:], W(l, WR1N), outt[g][:],
                                     start=False, stop=True)
                    h = xo_pool.tile([D, SH], BF, name=f"hown_{l}_{g}",
                                     tag=f"xo{g}")
                    if l == 0:
                        nc.vector.tensor_scalar(h[:], psa[g][:], B(l, BL),
                                                0.0, ADD, MAX)
                    else:
                        nc.vector.tensor_scalar(h[:], psa[g][:], B(l, BL),
                                                None, ADD)
                    hT[g] = h
                    stage = agb_in[l] if g == 0 else agc_in[l]
                    if l == 0:
                        ownparts(g, stage)
                    else:
                        nc.gpsimd.dma_start(stage.ap()[0], h[:])
                    src_t = agb_in[l] if g == 0 else agc_in[l]
                    dst_t = agb_out[l] if g == 0 else agc_out[l]
                    nc.gpsimd.collective_compute(
                        "AllGather", mybir.AluOpType.bypass,
                        replica_groups=rg,
                        ins=[src_t[:]], outs=[dst_t[:]])

                # ================= layer emission =================
                php[0] = pb_pool.tile([128, 512], F32, tag="php",
                                      name=f"php_{l}_0")
                csa[0] = cs_pool.tile([128, 1024], BF, tag="csa",
                                      name=f"csa_{l}_0")
                csa[1] = cs_pool.tile([128, 1024], BF, tag="csa",
                                      name=f"csa_{l}_1")

                if l == 0:
                    for c in range(8):
                        proj_chunk(0, c)
                    proj_q(0)
                    for jb in range(8):
                        yfill(0, jb)
                    # second-half projections interleave into pairs 6..13
                    ins2 = []
                    for c in range(8):
                        ins2.append(lambda c=c: proj_chunk(1, c))
                        ins2.append(lambda c=c: yfill(1, c))
                else:
                    gather_kvy(0)
                    proj_q(0)
                    ins2 = []

                # attention(g0): previous layer's g1 SAGE tail interleaved
                # (3 items/pair -- its deps precede this layer's start), plus
                # the half-1 projections for l=0
                for p in range(NPAIR):
                    attn_scores(0, p)
                    if len(pend[0]) > 2:
                        attn_value(0)
                    for _ in range(4):
                        if carry:
                            carry.pop(0)()
                    if p >= 6:
                        for _ in range(2):
                            if ins2:
                                ins2.pop(0)()
                    if p == 15:
                        proj_q(1)   # xo[1]: external (l0) / carried h (l1)
                        if l == 1:
                            gather_kvy(1)
                while pend[0]:
                    attn_value(0)
                epi(0)

                # attention(g1) with the g0 SAGE tail interleaved
                php[1] = pb_pool.tile([128, 512], F32, tag="php",
                                      name=f"php_{l}_1")
                fill = []
                of0 = [None]
                ub0 = ub_pool.tile([128, N], BF, tag="ub", name=f"ub_{l}_0")

                def mk_uf(jb):
                    return lambda: ufill(0, of0, ub0, jb)

                def mk_sub(jb):
                    return lambda: vsub(0, ub0, jb)

                def mk_mf(jb):
                    return lambda: mfill(0, jb)

                for jb in range(8):
                    fill.append(mk_uf(jb))
                    fill.append(mk_sub(jb))
                for jb in range(8):
                    fill.append(mk_mf(jb))

                # AG-A absorbs the residual inter-core skew on layer 0, so
                # its result lands much later there -- start the g0-tail
                # fillers later to keep the in-order PE queue from stalling
                f_start = 26 if l == 0 else 13
                for p in range(NPAIR):
                    attn_scores(1, p)
                    if len(pend[1]) > 2:
                        attn_value(1)
                    if p == f_start - 2:
                        of0[0] = gather_of(0)
                    if p >= f_start:
                        for _ in range(3 if l == 0 else 2):
                            if fill:
                                fill.pop(0)()
                while pend[1]:
                    attn_value(1)
                epi(1)
                while fill:
                    fill.pop(0)()
                tail_end(0)                       # issues AG-B
                if l == 1:
                    xg_next[0] = gather_x(0)
                of1 = [gather_of(1)]

                # g1 SAGE tail -> carried into the next phase's PE stream.
                # NOTE: bind every per-layer name via default args -- the
                # loop body is one scope and rebinds them next iteration.
                ub1 = ub_pool.tile([128, N], BF, tag="ub", name=f"ub_{l}_1")

                def mk_uf1(jb, f=ufill, o=of1, u=ub1):
                    return lambda: f(1, o, u, jb)

                def mk_sub1(jb, f=vsub, u=ub1):
                    return lambda: f(1, u, jb)

                def mk_mf1(jb, f=mfill):
                    return lambda: f(1, jb)

                carry = []
                for jb in range(8):
                    carry.append(mk_uf1(jb))
                    carry.append(mk_sub1(jb))
                for jb in range(8):
                    carry.append(mk_mf1(jb))
                carry.append(lambda f=tail_end: f(1))
                if l == 1:
                    carry.append(
                        lambda xn=xg_next, f=gather_x: xn.__setitem__(
                            1, f(1)))

                if l == 0:
                    xo = hT
                else:
                    hown_final = hT
                    xgt2 = xg_next

            # ---- final adjacency: sigmoid(F @ F^T), own 1024 rows ----
            def final_block(ch, grow, rt, cp):
                lhs = hown_final[grow][:, rt * 128:(rt + 1) * 128]
                ps_z = pa_pool.tile([128, 1024], F32, tag="ps_s",
                                    name=f"psz_{ch}_{grow}_{rt}_{cp}")
                for i in range(2):
                    c0 = cp * 1024 + i * 512
                    nc.tensor.matmul(
                        ps_z[:, i * 512:(i + 1) * 512], lhs,
                        xgt2[ch][:, c0:c0 + 512],
                        start=True, stop=True)
                z = z_pool.tile([128, 1024], FP16, tag="z",
                                name=f"z_{ch}_{grow}_{rt}_{cp}")
                nc.scalar.activation(
                    z[:], ps_z[:], mybir.ActivationFunctionType.Sigmoid)
                nc.sync.dma_start(
                    out_ext[grow, rt * 128:(rt + 1) * 128,
                            ch * N + cp * 1024:ch * N + (cp + 1) * 1024],
                    z[:])

            def final_step(ch, grow, rt, cp):
                final_block(ch, grow, rt, cp)
                for _ in range(5):
                    if carry:
                        carry.pop(0)()

            blocks = [(ch, grow, rt, cp) for ch in range(2)
                      for grow in range(2) for rt in range(4)
                      for cp in range(4)]
            for blk in blocks:
                final_step(*blk)

    nc.compile()
    return nc


def _host_prep(inputs):
    """Build per-core input maps from the full problem inputs."""
    x1 = np.asarray(inputs["x1"], np.float32)
    x2 = np.asarray(inputs["x2"], np.float32)
    x1t = np.ascontiguousarray(x1.T).astype(BF16)
    x2t = np.ascontiguousarray(x2.T).astype(BF16)

    def norm_adj_t(ei):
        ei = np.asarray(ei)
        A = np.zeros((N, N), np.float32)
        np.add.at(A, (ei[1], ei[0]), 1.0)
        deg = A.sum(1)
        A /= np.maximum(deg, 1.0)[:, None]
        return np.ascontiguousarray(A.T)  # MT[j, n]

    mt = [norm_adj_t(inputs["ei1"]), norm_adj_t(inputs["ei2"])]

    wm = np.zeros((14, 128, 128), np.float32)
    bs = np.zeros((8, 128, 1), np.float32)
    for l, s in enumerate(("1", "2")):
        wm[7 * l + WK] = inputs["Wk" + s]
        wm[7 * l + WQ] = inputs["Wq" + s]
        wm[7 * l + WV] = inputs["Wv" + s]
        wm[7 * l + WLS] = inputs["Wl" + s][:128] + inputs["Wl" + s][128:]
        wm[7 * l + WL1N] = -inputs["Wl" + s][128:]
        wm[7 * l + WRS] = inputs["Wr" + s][:128] + inputs["Wr" + s][128:]
        wm[7 * l + WR1N] = -inputs["Wr" + s][128:]
        bs[4 * l + BK, :, 0] = inputs["bk" + s]
        bs[4 * l + BQ, :, 0] = inputs["bq" + s]
        bs[4 * l + BV, :, 0] = inputs["bv" + s]
        bs[4 * l + BL, :, 0] = inputs["bl" + s]
    wm = wm.astype(BF16)

    in_maps = []
    for c in range(NCORES):
        sl = slice(c * SH, (c + 1) * SH)
        in_maps.append({
            "x1t": x1t,
            "x2t": x2t,
            "xown": np.stack([x1t[:, sl], x2t[:, sl]]),
            "mtc1": np.ascontiguousarray(
                mt[0][:, sl].astype(BF16).reshape(NJ, 128, SH)),
            "mtc2": np.ascontiguousarray(
                mt[1][:, sl].astype(BF16).reshape(NJ, 128, SH)),
            "wm": wm,
            "bs": bs,
        })
    return in_maps


def _assemble(results):
    full = np.empty((2 * N, 2 * N), np.float32)
    for c in range(NCORES):
        o = results[c]["out"]
        full[c * SH:(c + 1) * SH] = o[0]
        full[N + c * SH:N + (c + 1) * SH] = o[1]
    return full


def get_nc():
    if "nc" not in _cache:
        _cache["nc"] = _build_nc()
    return _cache["nc"]


def kernel(**inputs):
    nc = get_nc()
    in_maps = _host_prep(inputs)
    res = run_bass_kernel_spmd(nc, in_maps, core_ids=list(range(NCORES)))
    return _assemble(res.results)


# revision 38
# speedup vs baseline: 1.0533x; 1.0011x over previous
"""Trainium2 Bass kernel for nn_AttMatch (2-graph attention + SAGEConv GNN).

Self-contained: takes the full unsharded inputs of the reference problem,
shards across 8 NeuronCores internally, runs one SPMD NEFF, and gathers the
full [8192, 8192] sigmoid adjacency output.

Sharding (v5): query-sharded attention -- each core owns a 512-node slice of
EACH graph; keys/values cover the full 8192-row concatenated target set on
every core, so the softmax over targets is fully local (no AllReduce).

Collectives per layer (AllGather only, warmed up at start):
  A: out(g0)
  B (l=0): k|v|y own-slices for layer 1 (g0, computed with layer-1 weights
           right after h(g0)) merged with out(g1)    -> layer 1 needs no
  C (l=0): k|v|y own-slices (g1)                        projections at all
  B (l=1): h(g0) | out(g1)   (final-phase columns)
  C (l=1): h(g1)

SAGEConv is applied once per (layer, graph) via the algebraic merge
  h = relu( M @ (X@Wls - out@Wl1) + X_own@Wrs - out_own@Wr1 + bl ),
with M^T column shards loaded once (M is layer-invariant).  The g0 SAGE tail
is interleaved into attention(g1); the g1 tail is carried into the next
phase's PE stream.  The attention inner loop runs with a 2-pair lookahead so
the in-order PE queue never waits on exp or the epilogue.  Output in fp16.
"""

import numpy as np
import ml_dtypes

import concourse.bass as bass
import concourse.bacc as bacc
import concourse.tile as tile
import concourse.mybir as mybir
from concourse.bass_utils import run_bass_kernel_spmd

BF16 = ml_dtypes.bfloat16

N = 4096           # nodes per graph
D = 128            # feature dim (in == out == 128)
NCORES = 8
SH = N // NCORES   # 512-node own slice per graph per core
T = 2 * N          # concatenated target set
NTT = T // 128     # 64 target tiles
NPAIR = NTT // 2   # 32 pairs of target tiles (one exp per pair)
NJ = N // 128      # 32 source-node tiles per graph
INV_SCALE = 1.0 / np.sqrt(128.0)

F32 = mybir.dt.float32
BF = mybir.dt.bfloat16
FP16 = mybir.dt.float16

ADD = mybir.AluOpType.add
MULT = mybir.AluOpType.mult
MAX = mybir.AluOpType.max

# wm indices (per layer l: base = 7*l)
WK, WQ, WV, WLS, WL1N, WRS, WR1N = range(7)
# bias indices (per layer l: base = 4*l)
BK, BQ, BV, BL = range(4)

_cache = {}


def _build_nc():
    nc = bacc.Bacc("TRN2", target_bir_lowering=False, debug=False,
                   num_devices=NCORES)

    # ---- external I/O ----
    x1t = nc.dram_tensor("x1t", [D, N], BF, kind="ExternalInput")
    x2t = nc.dram_tensor("x2t", [D, N], BF, kind="ExternalInput")
    xgt_in = [x1t, x2t]
    xown_in = nc.dram_tensor("xown", [2, D, SH], BF, kind="ExternalInput")
    mtc_in = [nc.dram_tensor("mtc1", [NJ, 128, SH], BF, kind="ExternalInput"),
              nc.dram_tensor("mtc2", [NJ, 128, SH], BF, kind="ExternalInput")]
    wm_in = nc.dram_tensor("wm", [14, 128, 128], BF, kind="ExternalInput")
    bs_in = nc.dram_tensor("bs", [8, 128, 1], F32, kind="ExternalInput")
    out_ext = nc.dram_tensor("out", [2, SH, T], FP16, kind="ExternalOutput")

    # ---- internal DRAM for collectives ----
    rg = [list(range(NCORES))]
    ago_in = [nc.dram_tensor(f"ago_in_{l}", [D, SH], BF) for l in range(2)]
    ago_out = [nc.dram_tensor(f"ago_out_{l}", [NCORES, D, SH], BF,
                              addr_space="Shared") for l in range(2)]
    # AG-B payloads: l=0 -> [k0, v0, y0, out1]; l=1 -> [h0, out1]
    agb_n = [4, 2]
    agb_in = [nc.dram_tensor(f"agb_in_{l}", [agb_n[l], D, SH], BF)
              for l in range(2)]
    agb_out = [nc.dram_tensor(f"agb_out_{l}", [NCORES, agb_n[l], D, SH], BF,
                              addr_space="Shared") for l in range(2)]
    # AG-C payloads: l=0 -> [k1, v1, y1]; l=1 -> [h1]
    agc_n = [3, 1]
    agc_in = [nc.dram_tensor(f"agc_in_{l}", [agc_n[l], D, SH], BF)
              for l in range(2)]
    agc_out = [nc.dram_tensor(f"agc_out_{l}", [NCORES, agc_n[l], D, SH], BF,
                              addr_space="Shared") for l in range(2)]

    with tile.TileContext(nc) as tc:
        with (
            tc.tile_pool(name="const", bufs=1) as cpool,
            tc.tile_pool(name="xt", bufs=1) as xt_pool,
            tc.tile_pool(name="xo", bufs=2) as xo_pool,
            tc.tile_pool(name="kv", bufs=1) as kv_pool,
            tc.tile_pool(name="qt", bufs=1) as qt_pool,
            tc.tile_pool(name="es", bufs=5) as es_pool,
            tc.tile_pool(name="cs", bufs=2) as cs_pool,
            tc.tile_pool(name="st", bufs=2) as st_pool,
            tc.tile_pool(name="kvo", bufs=1) as kvo_pool,
            tc.tile_pool(name="y", bufs=1) as y_pool,
            tc.tile_pool(name="ub", bufs=1) as ub_pool,
            tc.tile_pool(name="of", bufs=1) as of_pool,
            tc.tile_pool(name="mt", bufs=16) as mt_pool,
            tc.tile_pool(name="z", bufs=3) as z_pool,
            tc.tile_pool(name="pa", bufs=2, space="PSUM") as pa_pool,
            tc.tile_pool(name="pb", bufs=1, space="PSUM") as pb_pool,
            tc.tile_pool(name="pc", bufs=2, space="PSUM") as pc_pool,
        ):
            # ---- constants ----
            wm = cpool.tile([128, 14 * 128], BF, name="wm_sb")
            nc.sync.dma_start(
                wm.rearrange("p (i f) -> p i f", i=14),
                wm_in.ap().rearrange("i p f -> p i f"))
            bs = cpool.tile([128, 8], F32, name="bs_sb")
            nc.sync.dma_start(
                bs.rearrange("p (i f) -> p i f", i=8),
                bs_in.ap().rearrange("i p f -> p i f"))
            ones_m1 = cpool.tile([128, 1], BF, name="ones_m1")
            nc.vector.memset(ones_m1[:], 1.0)
            ones_c = cpool.tile([1, 128], BF, name="ones_c")
            nc.vector.memset(ones_c[:], 1.0)

            def W(l, i):
                base = 7 * l + i
                return wm[:, 128 * base:128 * (base + 1)]

            def B(l, i):
                return bs[:, 4 * l + i:4 * l + i + 1]

            # warm up the collective rings FIRST (before any bulk load) so
            # the first-CC cold-start / inter-core skew is absorbed as early
            # as possible
            wc_in = nc.dram_tensor("wc_in", [128, 8], BF)
            wc_out = nc.dram_tensor("wc_out", [NCORES, 128, 8], BF,
                                    addr_space="Shared")
            nc.sync.dma_start(wc_in.ap(), wm[:, 0:8])
            nc.gpsimd.collective_compute(
                "AllGather", mybir.AluOpType.bypass, replica_groups=rg,
                ins=[wc_in.ap()], outs=[wc_out.ap()])

            # ---- generation-0 node features (split loads: compute starts
            # after the first chunk) ----
            xg = []
            for g in range(2):
                t = xt_pool.tile([D, N], BF, name=f"x{g}t_0", tag=f"xt{g}")
                for hh in range(2):
                    nc.sync.dma_start(t[:, hh * 2048:(hh + 1) * 2048],
                                      xgt_in[g][:, hh * 2048:(hh + 1) * 2048])
                xg.append(t)
            xo = []
            for g in range(2):
                t = xo_pool.tile([D, SH], BF, name=f"xown{g}_0", tag=f"xo{g}")
                nc.sync.dma_start(t[:], xown_in[g])
                xo.append(t)

            # M^T column shards: identical for both layers -- load once and
            # keep resident (16 tiles, never rewritten)
            mtcs = [[], []]
            for g in range(2):
                for jb in range(8):
                    mtc_t = mt_pool.tile([128, 4 * SH], BF, tag="mt",
                                         name=f"mtc_{g}_{jb}")
                    nc.sync.dma_start(
                        mtc_t.rearrange("p (j n) -> p j n", j=4),
                        mtc_in[g].ap()[4 * jb:4 * jb + 4]
                        .rearrange("j p n -> p j n"))
                    mtcs[g].append(mtc_t)

            hown_final = None
            xgt2 = [None, None]
            carry = []          # g1 SAGE tail of the previous layer

            for l in range(2):
                kt = kv_pool.tile([D, T], BF, name=f"kt_{l}", tag="kt")
                vn = kv_pool.tile([128, T], BF, name=f"vn_{l}", tag="vn")
                ybig = [y_pool.tile([128, N], BF, name=f"y_{l}_{g}",
                                    tag=f"y{g}") for g in range(2)]
                qt = [None, None]
                csa = [None, None]
                php = [None, None]
                outt = [None, None]
                psa = [None, None]
                hT = [None, None]
                xg_next = [None, None]
                pend = {0: [], 1: []}
                xg_l = xg
                xo_l = xo

                def proj_chunk(h, c, l=l, kt=kt, vn=vn, xg_l=xg_l):
                    """k^T and natural-layout v tiles, one 512-wide chunk of
                    target half h (layer 0 only; layer 1 gets k/v/y via AG)."""
                    ps = pc_pool.tile([128, 512], F32, tag="c",
                                      name=f"psk_{l}_{h}_{c}")
                    nc.tensor.matmul(ps[:], W(l, WK),
                                     xg_l[h][:, c * 512:(c + 1) * 512],
                                     start=True, stop=True)
                    nc.vector.tensor_scalar(
                        kt[:, h * N + c * 512:h * N + (c + 1) * 512],
                        ps[:], B(l, BK), None, ADD)
                    psv = pc_pool.tile([128, 512], F32, tag="c",
                                       name=f"psv_{l}_{h}_{c}")
                    for k in range(4):
                        t0 = c * 4 + k
                        nc.tensor.matmul(
                            psv[:, k * 128:(k + 1) * 128],
                            xg_l[h][:, t0 * 128:(t0 + 1) * 128],
                            W(l, WV), start=True, stop=True)
                    # v bias folded in post-softmax (sum(alpha)=1)
                    nc.vector.tensor_copy(
                        vn[:, h * N + c * 512:h * N + (c + 1) * 512],
                        psv[:])

                def gather_kvy(h, l=l, kt=kt, vn=vn, ybig=ybig):
                    """Layer 1: k/v/y for target half h arrive via AG-B/C of
                    layer 0 (computed there with layer-1 weights).  k and v
                    are gathered in core-block halves so the first attention
                    pairs unblock before the full gather lands."""
                    src = agb_out[0].ap() if h == 0 else agc_out[0].ap()
                    for i, dst in ((0, kt[:, h * N:(h + 1) * N]),
                                   (1, vn[:, h * N:(h + 1) * N])):
                        for q in range(2):
                            cs_ = slice(q * 4, (q + 1) * 4)
                            nc.sync.dma_start(
                                dst[:, q * 2048:(q + 1) * 2048]
                                .rearrange("p (c n) -> p c n", c=4),
                                src[cs_, i].rearrange("c p n -> p c n"))
                    nc.sync.dma_start(
                        ybig[h].rearrange("p (c n) -> p c n", c=NCORES),
                        src[:, 2].rearrange("c p n -> p c n"))

                def proj_q(g, l=l, qt=qt, xo_l=xo_l):
                    ps = pc_pool.tile([128, 512], F32, tag="c",
                                      name=f"psq_{l}_{g}")
                    nc.tensor.matmul(ps[:], W(l, WQ), xo_l[g][:],
                                     start=True, stop=True)
                    q = qt_pool.tile([D, SH], BF, name=f"qt_{l}_{g}",
                                     tag=f"qt{g}")
                    nc.vector.tensor_scalar(q[:], ps[:], B(l, BQ), None, ADD)
                    qt[g] = q

                def yfill(g, jb, l=l, ybig=ybig, xg_l=xg_l):
                    """Y = X @ Wls, natural layout, 4 node tiles (layer 0)."""
                    psy = pc_pool.tile([128, 512], F32, tag="c",
                                       name=f"psy_{l}_{g}_{jb}")
                    for k in range(4):
                        jt = jb * 4 + k
                        nc.tensor.matmul(psy[:, k * 128:(k + 1) * 128],
                                         xg_l[g][:, jt * 128:(jt + 1) * 128],
                                         W(l, WLS), start=True, stop=True)
                    nc.vector.tensor_copy(
                        ybig[g][:, jb * 512:(jb + 1) * 512], psy[:])

                def attn_scores(g, p, l=l, kt=kt, qt=qt, pend=pend):
                    ps_s = pa_pool.tile([128, 1024], F32, tag="ps_s",
                                        name=f"pss_{l}_{g}_{p}")
                    for i in range(2):
                        tt = 2 * p + i
                        nc.tensor.matmul(ps_s[:, i * 512:(i + 1) * 512],
                                         kt[:, tt * 128:(tt + 1) * 128],
                                         qt[g][:], start=True, stop=True)
                    es = es_pool.tile([128, 1024], BF, tag="es",
                                      name=f"es_{l}_{g}_{p}")
                    nc.scalar.activation(es[:], ps_s[:],
                                         mybir.ActivationFunctionType.Exp,
                                         scale=INV_SCALE)
                    pend[g].append((p, es))

                def attn_value(g, l=l, vn=vn, php=php, csa=csa, pend=pend):
                    p, es = pend[g].pop(0)
                    for i in range(2):
                        tt = 2 * p + i
                        nc.tensor.matmul(php[g][:],
                                         vn[:, tt * 128:(tt + 1) * 128],
                                         es[:, i * 512:(i + 1) * 512],
                                         start=(tt == 0), stop=(tt == NTT - 1))
                    if p == 0:
                        nc.vector.tensor_copy(csa[g][:], es[:])
                    else:
                        nc.vector.tensor_tensor(csa[g][:], csa[g][:], es[:],
                                                ADD)

                def epi(g, l=l, csa=csa, php=php, outt=outt):
                    """softmax epilogue: outT = php / colsum + bv.

                    g0 publishes via AG-A; g1 stages into the AG-B payload
                    (published together with the g0 tail)."""
                    csf = st_pool.tile([128, 512], BF, tag="csf",
                                       name=f"csf_{l}_{g}")
                    nc.vector.tensor_tensor(csf[:], csa[g][:, 0:512],
                                            csa[g][:, 512:1024], ADD)
                    pcs = pc_pool.tile([1, 512], F32, tag="c",
                                       name=f"pcs_{l}_{g}")
                    nc.tensor.matmul(pcs[:], ones_m1[:], csf[:],
                                     start=True, stop=True)
                    csb = st_pool.tile([1, 512], BF, tag="csb",
                                       name=f"csb_{l}_{g}")
                    nc.vector.tensor_copy(csb[:], pcs[:])
                    ps_rep = pc_pool.tile([128, 512], F32, tag="c",
                                          name=f"psrep_{l}_{g}")
                    nc.tensor.matmul(ps_rep[:], ones_c[:], csb[:],
                                     start=True, stop=True)
                    rr = st_pool.tile([128, 512], F32, tag="rr",
                                      name=f"rr_{l}_{g}")
                    nc.vector.reciprocal_approx_fast(rr[:], ps_rep[:])
                    pre = st_pool.tile([128, 512], F32, tag="pre",
                                       name=f"pre_{l}_{g}")
                    nc.vector.tensor_tensor(pre[:], php[g][:], rr[:], MULT)
                    ot = st_pool.tile([D, SH], BF, tag=f"ot{g}",
                                      name=f"outt_{l}_{g}")
                    nc.vector.tensor_scalar(ot[:], pre[:], B(l, BV), None,
                                            ADD)
                    outt[g] = ot
                    if g == 0:
                        nc.sync.dma_start(ago_in[l][:], ot[:])
                        nc.gpsimd.collective_compute(
                            "AllGather", mybir.AluOpType.bypass,
                            replica_groups=rg,
                            ins=[ago_in[l][:]], outs=[ago_out[l][:]])
                    else:
                        nc.sync.dma_start(
                            agb_in[l].ap()[agb_n[l] - 1], ot[:])

                def gather_of(g, l=l):
                    # post-collective gathers go on the gpsimd queue so the
                    # CC-semaphore wait never blocks the sync DMA queue
                    of = of_pool.tile([D, N], BF, tag="of",
                                      name=f"of_{l}_{g}")
                    src = (ago_out[l].ap() if g == 0
                           else agb_out[l].ap()[:, agb_n[l] - 1])
                    nc.sync.dma_start(
                        of.rearrange("p (c n) -> p c n", c=NCORES),
                        src.rearrange("c p n -> p c n"))
                    return of

                def gather_x(g, l=l):
                    # layer 1 only: the full h (= final-phase F^T columns)
                    t = xt_pool.tile([D, N], BF, name=f"x{g}t_{l + 1}",
                                     tag=f"xt{g}")
                    src = (agb_out[l].ap()[:, 0] if g == 0
                           else agc_out[l].ap()[:, 0])
                    nc.sync.dma_start(
                        t.rearrange("p (c n) -> p c n", c=NCORES),
                        src.rearrange("c p n -> p c n"))
                    return t

                def ufill(g, ofl, ub, jb, l=l):
                    """U' = out @ (-Wl1), natural layout, 4 node tiles."""
                    psu = pc_pool.tile([128, 512], F32, tag="c",
                                       name=f"psu_{l}_{g}_{jb}")
                    for k in range(4):
                        jt = jb * 4 + k
                        nc.tensor.matmul(psu[:, k * 128:(k + 1) * 128],
                                         ofl[0][:, jt * 128:(jt + 1) * 128],
                                         W(l, WL1N), start=True, stop=True)
                    nc.vector.tensor_copy(ub[:, jb * 512:(jb + 1) * 512],
                                          psu[:])

                def vsub(g, ub, jb, ybig=ybig):
                    # V = Y + U', one 512-wide chunk (in place over ybig) so
                    # each M group unblocks as soon as its chunk is ready
                    sl = slice(jb * 512, (jb + 1) * 512)
                    nc.vector.tensor_tensor(ybig[g][:, sl], ybig[g][:, sl],
                                            ub[:, sl], ADD)

                def mfill(g, jb, l=l, psa=psa, ybig=ybig):
                    if psa[g] is None:
                        psa[g] = pb_pool.tile([128, 512], F32, tag="psa",
                                              name=f"psa_{l}_{g}")
                    for k in range(4):
                        jt = jb * 4 + k
                        nc.tensor.matmul(psa[g][:],
                                         ybig[g][:, jt * 128:(jt + 1) * 128],
                                         mtcs[g][jb][:, k * SH:(k + 1) * SH],
                                         start=(jt == 0), stop=False)

                def ownparts(g, stage, l=l, hT=hT):
                    """Layer 0 tails: produce layer-1 k/v/y for the own node
                    slice (from h) and stage them into the AG payload."""
                    h = hT[g]
                    psk = pc_pool.tile([128, 512], F32, tag="c",
                                       name=f"psok_{l}_{g}")
                    nc.tensor.matmul(psk[:], W(1, WK), h[:],
                                     start=True, stop=True)
                    ko = kvo_pool.tile([D, 3 * SH], BF, tag=f"ko{g}",
                                       name=f"ko_{l}_{g}")
                    nc.vector.tensor_scalar(ko[:, 0:512], psk[:], B(1, BK),
                                            None, ADD)
                    psv = pc_pool.tile([128, 512], F32, tag="c",
                                       name=f"psov_{l}_{g}")
                    for k in range(4):
                        nc.tensor.matmul(psv[:, k * 128:(k + 1) * 128],
                                         h[:, k * 128:(k + 1) * 128],
                                         W(1, WV), start=True, stop=True)
                    nc.vector.tensor_copy(ko[:, 512:1024], psv[:])
                    psy = pc_pool.tile([128, 512], F32, tag="c",
                                       name=f"psoy_{l}_{g}")
                    for k in range(4):
                        nc.tensor.matmul(psy[:, k * 128:(k + 1) * 128],
                                         h[:, k * 128:(k + 1) * 128],
                                         W(1, WLS), start=True, stop=True)
                    nc.vector.tensor_copy(ko[:, 1024:1536], psy[:])
                    nc.sync.dma_start(
                        stage.ap()[0:3].rearrange("i p n -> p i n"),
                        ko.rearrange("p (i n) -> p i n", i=3))

                def tail_end(g, l=l, psa=psa, outt=outt, hT=hT, xo_l=xo_l,
                             ownparts=ownparts):
                    """Wr terms + bias(+relu); publish AG-B (g0) / AG-C (g1)."""
                    nc.tensor.matmul(psa[g][:], W(l, WRS), xo_l[g][:],
                                     start=False, stop=False)
                    nc.tensor.matmul(psa[g][:], W(l, WR1N), outt[g][:],
                                     start=False, stop=True)
                    h = xo_pool.tile([D, SH], BF, name=f"hown_{l}_{g}",
                                     tag=f"xo{g}")
                    if l == 0:
                        nc.vector.tensor_scalar(h[:], psa[g][:], B(l, BL),
                                                0.0, ADD, MAX)
                    else:
                        nc.vector.tensor_scalar(h[:], psa[g][:], B(l, BL),
                                                None, ADD)
                    hT[g] = h
                    stage = agb_in[l] if g == 0 else agc_in[l]
                    if l == 0:
                        ownparts(g, stage)
                    else:
                        nc.sync.dma_start(stage.ap()[0], h[:])
                    src_t = agb_in[l] if g == 0 else agc_in[l]
                    dst_t = agb_out[l] if g == 0 else agc_out[l]
                    nc.gpsimd.collective_compute(
                        "AllGather", mybir.AluOpType.bypass,
                        replica_groups=rg,
                        ins=[src_t[:]], outs=[dst_t[:]])

                # ================= layer emission =================
                php[0] = pb_pool.tile([128, 512], F32, tag="php",
                                      name=f"php_{l}_0")
                csa[0] = cs_pool.tile([128, 1024], BF, tag="csa",
                                      name=f"csa_{l}_0")
                csa[1] = cs_pool.tile([128, 1024], BF, tag="csa",
                                      name=f"csa_{l}_1")

                if l == 0:
                    for c in range(8):
                        proj_chunk(0, c)
                    proj_q(0)
                    for jb in range(8):
                        yfill(0, jb)
                    # second-half projections interleave into pairs 6..13
                    ins2 = []
                    for c in range(8):
                        ins2.append(lambda c=c: proj_chunk(1, c))
                        ins2.append(lambda c=c: yfill(1, c))
                else:
                    gather_kvy(0)
                    proj_q(0)
                    ins2 = []

                # attention(g0): previous layer's g1 SAGE tail interleaved
                # (3 items/pair -- its deps precede this layer's start), plus
                # the half-1 projections for l=0
                for p in range(NPAIR):
                    attn_scores(0, p)
                    if len(pend[0]) > 2:
                        attn_value(0)
                    for _ in range(4):
                        if carry:
                            carry.pop(0)()
                    if p >= 6:
                        for _ in range(2):
                            if ins2:
                                ins2.pop(0)()
                    if p == 15:
                        proj_q(1)   # xo[1]: external (l0) / carried h (l1)
                        if l == 1:
                            gather_kvy(1)
                while pend[0]:
                    attn_value(0)
                epi(0)

                # attention(g1) with the g0 SAGE tail interleaved
                php[1] = pb_pool.tile([128, 512], F32, tag="php",
                                      name=f"php_{l}_1")
                fill = []
                of0 = [None]
                ub0 = ub_pool.tile([128, N], BF, tag="ub", name=f"ub_{l}_0")

                def mk_uf(jb):
                    return lambda: ufill(0, of0, ub0, jb)

                def mk_sub(jb):
                    return lambda: vsub(0, ub0, jb)

                def mk_mf(jb):
                    return lambda: mfill(0, jb)

                for jb in range(8):
                    fill.append(mk_uf(jb))
                    fill.append(mk_sub(jb))
                for jb in range(8):
                    fill.append(mk_mf(jb))

                # AG-A absorbs the residual inter-core skew on layer 0, so
                # its result lands much later there -- start the g0-tail
                # fillers later to keep the in-order PE queue from stalling
                f_start = 26 if l == 0 else 13
                for p in range(NPAIR):
                    attn_scores(1, p)
                    if len(pend[1]) > 2:
                        attn_value(1)
                    if p == f_start - 2:
                        of0[0] = gather_of(0)
                    if p >= f_start:
                        for _ in range(3 if l == 0 else 2):
                            if fill:
                                fill.pop(0)()
                while pend[1]:
                    attn_value(1)
                epi(1)
                while fill:
                    fill.pop(0)()
                tail_end(0)                       # issues AG-B
                if l == 1:
                    xg_next[0] = gather_x(0)
                of1 = [gather_of(1)]

                # g1 SAGE tail -> carried into the next phase's PE stream.
                # NOTE: bind every per-layer name via default args -- the
                # loop body is one scope and rebinds them next iteration.
                ub1 = ub_pool.tile([128, N], BF, tag="ub", name=f"ub_{l}_1")

                def mk_uf1(jb, f=ufill, o=of1, u=ub1):
                    return lambda: f(1, o, u, jb)

                def mk_sub1(jb, f=vsub, u=ub1):
                    return lambda: f(1, u, jb)

                def mk_mf1(jb, f=mfill):
                    return lambda: f(1, jb)

                carry = []
                for jb in range(8):
                    carry.append(mk_uf1(jb))
                    carry.append(mk_sub1(jb))
                for jb in range(8):
                    carry.append(mk_mf1(jb))
                carry.append(lambda f=tail_end: f(1))
                if l == 1:
                    carry.append(
                        lambda xn=xg_next, f=gather_x: xn.__setitem__(
                            1, f(1)))

                if l == 0:
                    xo = hT
                else:
                    hown_final = hT
                    xgt2 = xg_next

            # ---- final adjacency: sigmoid(F @ F^T), own 1024 rows ----
            def final_block(ch, grow, rt, cp):
                lhs = hown_final[grow][:, rt * 128:(rt + 1) * 128]
                ps_z = pa_pool.tile([128, 1024], F32, tag="ps_s",
                                    name=f"psz_{ch}_{grow}_{rt}_{cp}")
                for i in range(2):
                    c0 = cp * 1024 + i * 512
                    nc.tensor.matmul(
                        ps_z[:, i * 512:(i + 1) * 512], lhs,
                        xgt2[ch][:, c0:c0 + 512],
                        start=True, stop=True)
                z = z_pool.tile([128, 1024], FP16, tag="z",
                                name=f"z_{ch}_{grow}_{rt}_{cp}")
                nc.scalar.activation(
                    z[:], ps_z[:], mybir.ActivationFunctionType.Sigmoid)
                nc.sync.dma_start(
                    out_ext[grow, rt * 128:(rt + 1) * 128,
                            ch * N + cp * 1024:ch * N + (cp + 1) * 1024],
                    z[:])

            def final_step(ch, grow, rt, cp):
                final_block(ch, grow, rt, cp)
                for _ in range(5):
                    if carry:
                        carry.pop(0)()

            blocks = [(ch, grow, rt, cp) for ch in range(2)
                      for grow in range(2) for rt in range(4)
                      for cp in range(4)]
            for blk in blocks:
                final_step(*blk)

    nc.compile()
    return nc


def _host_prep(inputs):
    """Build per-core input maps from the full problem inputs."""
    x1 = np.asarray(inputs["x1"], np.float32)
    x2 = np.asarray(inputs["x2"], np.float32)
    x1t = np.ascontiguousarray(x1.T).astype(BF16)
    x2t = np.ascontiguousarray(x2.T).astype(BF16)

    def norm_adj_t(ei):
        ei = np.asarray(ei)
        A = np.zeros((N, N), np.float32)
        np.add.at(A, (ei[1], ei[0]), 1.0)
        deg = A.sum(1)
        A /= np.maximum(deg, 1.0)[:, None]
        return np.ascontiguousarray(A.T)  # MT[j, n]

    mt = [norm_adj_t(inputs["ei1"]), norm_adj_t(inputs["ei2"])]

    wm = np.zeros((14, 128, 128), np.float32)
    bs = np.zeros((8, 128, 1), np.float32)
    for l, s in enumerate(("1", "2")):
        wm[7 * l + WK] = inputs["Wk" + s]
        wm[7 * l + WQ] = inputs["Wq" + s]
        wm[7 * l + WV] = inputs["Wv" + s]
        wm[7 * l + WLS] = inputs["Wl" + s][:128] + inputs["Wl" + s][128:]
        wm[7 * l + WL1N] = -inputs["Wl" + s][128:]
        wm[7 * l + WRS] = inputs["Wr" + s][:128] + inputs["Wr" + s][128:]
        wm[7 * l + WR1N] = -inputs["Wr" + s][128:]
        bs[4 * l + BK, :, 0] = inputs["bk" + s]
        bs[4 * l + BQ, :, 0] = inputs["bq" + s]
        bs[4 * l + BV, :, 0] = inputs["bv" + s]
        bs[4 * l + BL, :, 0] = inputs["bl" + s]
    wm = wm.astype(BF16)

    in_maps = []
    for c in range(NCORES):
        sl = slice(c * SH, (c + 1) * SH)
        in_maps.append({
            "x1t": x1t,
            "x2t": x2t,
            "xown": np.stack([x1t[:, sl], x2t[:, sl]]),
            "mtc1": np.ascontiguousarray(
                mt[0][:, sl].astype(BF16).reshape(NJ, 128, SH)),
            "mtc2": np.ascontiguousarray(
                mt[1][:, sl].astype(BF16).reshape(NJ, 128, SH)),
            "wm": wm,
            "bs": bs,
        })
    return in_maps


def _assemble(results):
    full = np.empty((2 * N, 2 * N), np.float32)
    for c in range(NCORES):
        o = results[c]["out"]
        full[c * SH:(c + 1) * SH] = o[0]
        full[N + c * SH:N + (c + 1) * SH] = o[1]
    return full


def get_nc():
    if "nc" not in _cache:
        _cache["nc"] = _build_nc()
    return _cache["nc"]


def kernel(**inputs):
    nc = get_nc()
    in_maps = _host_prep(inputs)
    res = run_bass_kernel_spmd(nc, in_maps, core_ids=list(range(NCORES)))
    return _assemble(res.results)
